# revision 41
# baseline (speedup 1.0000x reference)
"""Trainium2 Bass kernel for KL-divergence 1-NN label lookup (AnchorStore).

reference:
    self[k]  = mean_d a[k,d]*log a[k,d]
    cross    = einsum('kd,bd->kb', a, log q) / D
    kl[b,k]  = self[k] - cross[k,b]
    out[b]   = queue_label[argmin_k kl[b,k]]

Strategy (8 NeuronCores, D-sharded, fp16 operands):
    Each core owns a D-slice (padded with 1.0 so log()=0 contributes
    nothing), shipped as fp16 in d-tile-major layout [128, NT, K].
    Working in SUM units (scale-invariant for argmin):
        m[b,k] = sum_d lq[d,b]*at[d,k] - sum_d at[d,k]*log(at[d,k])
    K is split into P=4 passes of KW columns; passes 0..P-3 share one
    merged ReduceScatter(add) and pass P-2 gets its own, both fully
    overlapped by compute, leaving only a small RS on the tail.
      - TensorE: stationary lq tiles [128d,128b] x moving at [128d,KW]
        accumulate cross into PSUM; the -self term accumulates via a
        (-1)-stationary x pair-summed t = at*log(at) (DVE adds d-tile
        pairs in fp16 to halve the self-matmul column count); those
        srep matmuls are deferred two batches behind their DVE
        producers to avoid tensor-queue head-of-line blocking.
      - ScalarE computes log() (Ln activation) in large batches.
      - Drain: m = pk + srep -> DRAM -> ReduceScatter.
    Pipeline ramp: pass 0 starts with small d-batches and pass 1's
    first two batches are interleaved into the ramp (they reuse the
    same lq tiles, and their PSUM parity banks are free) so the PE has
    a second stream of ready work while DMA/ACT latency fills.
    Tail: msum loads are pinned behind the last drain (WAW copy) so
    the scheduler can never hoist them; per-slice argmax partials
    (value + label via the is_equal trick) run under the last RS, and
    a 3-column combine emits 32 int32 labels per core; host concats.
"""

import os
import sys

import numpy as np

sys.path.insert(0, "/opt/trn_rl_repo")

from concourse import bacc, bass, mybir, tile  # noqa: E402
from concourse import bass_utils  # noqa: E402

K = 2048
B = 256
D = 50257
NCORES = 8
NT = 50             # d-tiles of 128 per core (padded)
DSH = NT * 128      # 6400
BS = B // NCORES    # 32 queries per core after ReduceScatter
F32 = mybir.dt.float32
F16 = mybir.dt.float16


def build(mm_dtype=F16, passes=4, bt=8, pair=True, warm_cc=True):
    """Build the SPMD Bass graph for one core (all cores identical)."""
    P = passes
    KW = K // P              # k columns per pass
    ncl = KW // 512 if KW >= 512 else 0   # full-512 chunks per pass
    assert KW % 512 == 0 or KW in (256,), KW
    nc = bacc.Bacc(
        "TRN2", target_bir_lowering=False, debug=False, num_devices=NCORES
    )
    # pass-major layout: each (pass, tile-range) batch is one fully
    # contiguous region per partition -> max DMA efficiency
    at_d = nc.dram_tensor(
        "at", [128, P, NT, KW], mm_dtype, kind="ExternalInput"
    )
    qt_d = nc.dram_tensor("qt", [128, NT, B], mm_dtype, kind="ExternalInput")
    lab_d = nc.dram_tensor("lab1", [BS, K], F32, kind="ExternalInput")
    out_d = nc.dram_tensor("out", [BS], mybir.dt.int32, kind="ExternalOutput")

    LN = mybir.ActivationFunctionType.Ln
    AX = mybir.AxisListType.X
    OP = mybir.AluOpType

    # d-tile batches (per pass): groups of `bt` tiles, even-sized for
    # pairs.  Pass 0 ramps up with small batches so the matmul pipeline
    # starts early; later passes use full batches (fewer overheads).
    def mk_batches(ramp):
        out = list(ramp)
        t0 = out[-1][1] if out else 0
        while t0 < NT:
            t1 = min(t0 + bt, NT)
            out.append((t0, t1))
            t0 = t1
        return out

    batches0 = mk_batches([(0, 2), (2, 6), (6, 14)])
    batches_rest = mk_batches([])

    # q chunks for lq computation (front chunks small for fast start)
    qch = [(0, 2), (2, 6), (6, 14), (14, 26), (26, 38), (38, NT)]

    with tile.TileContext(nc) as tc:
        with (
            tc.tile_pool(name="const", bufs=1) as constp,
            tc.tile_pool(name="lqp", bufs=1) as lqp,
            tc.tile_pool(name="qinp", bufs=2) as qinp,
            tc.tile_pool(name="atp", bufs=4) as atp,
            tc.tile_pool(name="latp", bufs=4) as latp,
            tc.tile_pool(name="ttp", bufs=4) as ttp,
            tc.tile_pool(name="tpp", bufs=6) as tpp,
            tc.tile_pool(name="msbp", bufs=2) as msbp,
            tc.tile_pool(name="epp", bufs=1) as epp,
            tc.tile_pool(name="psp", bufs=1, space="PSUM") as psp,
            tc.tile_pool(name="dramp", bufs=1, space="DRAM") as dramp,
        ):
            # --- constants / warmup ---------------------------------
            # Tiny dummy DMAs warm each HWDGE/SWDGE queue so the first
            # real input loads don't pay first-transfer setup latency.
            wdma_d = dramp.tile([1, 16], F32, name="wdma_d", bufs=1)
            for eng in (nc.sync, nc.scalar, nc.gpsimd):
                wdma_s = constp.tile([1, 16], F32, name=f"wdma_{eng.engine}")
                eng.dma_start(wdma_s[:], wdma_d[:])


            # --- lq = log(query^T), fp16, resident -------------------
            lq = lqp.tile([128, NT, B], mm_dtype)
            qsb = []
            for ci, (c0_, c1_) in enumerate(qch):
                qtile = qinp.tile(
                    [128, c1_ - c0_, B], mm_dtype, name=f"qtile_{ci}",
                    tag="qtile",
                )
                qsb.append((qtile, c0_, c1_))
            # All qt chunk DMAs are triggered up front on the (idle)
            # gpsimd queue so every chunk is in flight immediately --
            # the ~5us per-DMA completion latency then overlaps instead
            # of serialising with the lq activations.  The tiny dummy
            # Ln pulls the ACT table load forward, under the qt0 DMA.
            dum = constp.tile([128, 16], F32)
            nc.gpsimd.memset(dum[:], 1.0)
            dumo = constp.tile([128, 16], F32)
            nc.scalar.activation(dumo[:], dum[:], LN)
            negones_f = constp.tile([128, 128], F32)
            nc.gpsimd.memset(negones_f[:], -1.0)
            negones = constp.tile([128, 128], mm_dtype)
            nc.vector.tensor_copy(negones[:], negones_f[:])

            # qt triggers go first on gpsimd: nothing else on that
            # queue is needed before ~20us, and the first matmul waits
            # on qt chunk 0 -> lq.
            for qtile, c0_, c1_ in qsb[:3]:
                nc.gpsimd.dma_start(qtile[:], qt_d[:, c0_:c1_, :])
            nc.scalar.activation(
                lq[:, qch[0][0]:qch[0][1], :], qsb[0][0][:], LN
            )

            # ~40 dummy matmuls into a spare PSUM bank while the PE
            # waits for the first lq tile: the HAM clock gate needs
            # ~3.4us of sustained PE activity to lift the 1.2 GHz cold
            # throttle, so the first real matmuls start at full speed.
            warm_ps = psp.tile([128, 128], F32, name="warm_ps")
            for wi in range(40):
                nc.tensor.matmul(
                    warm_ps[:], negones[:], negones[:],
                    start=(wi == 0), stop=(wi == 39),
                )

            if warm_cc:
                # Tiny dummy collective: pre-warms ncfw/credit state on
                # the CC engine and doubles as the cross-core
                # rendezvous long before the first real ReduceScatter.
                w_in = dramp.tile([1, 64], F32)
                w_out = dramp.tile([NCORES, 64], F32)
                w_sb = constp.tile([1, 64], F32)
                nc.gpsimd.memset(w_sb[:], 1.0)
                nc.gpsimd.dma_start(w_in[:], w_sb[:])
                nc.gpsimd.collective_compute(
                    "AllGather",
                    OP.bypass,
                    replica_groups=[list(range(NCORES))],
                    ins=[w_in.opt()],
                    outs=[w_out.opt()],
                )

            lab1 = constp.tile([BS, K], F32)
            nc.gpsimd.dma_start(lab1[:], lab_d[:])

            # --- PSUM accumulators (parity double-buffered) ----------
            pk = {}
            srep = {}
            for par in range(min(2, P)):
                for bti in range(2):
                    for cl in range(max(1, ncl)):
                        pk[(par, bti, cl)] = psp.tile(
                            [128, min(KW, 512)], F32,
                            name=f"pk_{par}_{bti}_{cl}",
                            tag=f"pk_{par}_{bti}_{cl}",
                        )
                for cl in range(max(1, ncl)):
                    srep[(par, cl)] = psp.tile(
                        [128, min(KW, 512)], F32, name=f"srep_{par}_{cl}",
                        tag=f"srep_{par}_{cl}",
                    )

            # per-slice (value, label) partials, combined at the end:
            # slice 0 = merged passes [0, P-2), slice 1 = pass P-2,
            # slice 2 = pass P-1
            vcat = epp.tile([BS, 3], F32, bufs=1)
            lcat = epp.tile([BS, 3], F32, bufs=1)

            qt_dma_emitted = 1  # chunk 0 already emitted

            # Two collective groups: passes [0, P-1) share one big
            # ReduceScatter (launched after pass P-2, fully overlapped
            # by pass P-1 compute); the last pass gets a small RS on
            # the critical tail.  This keeps the CC engine far from
            # saturation and minimises tail latency.
            KWA = (P - 2) * KW
            ar_a = dramp.tile([B, KWA], F32, name="ar_a", bufs=1)
            rs_a = dramp.tile([BS, KWA], F32, name="rs_a", bufs=1)
            ar_c = dramp.tile([B, KW], F32, name="ar_c", bufs=1)
            rs_c = dramp.tile([BS, KW], F32, name="rs_c", bufs=1)
            ar_b = dramp.tile([B, KW], F32, name="ar_b", bufs=1)
            rs_b = dramp.tile([BS, KW], F32, name="rs_b", bufs=1)

            # Explicit (ps, bi) schedule: pass-1's first two batches
            # are interleaved into pass-0's ramp (they share lq tiles
            # and par=1 PSUM banks are free), so the tensor engine has
            # a second stream of ready work while pass-0's DMA/ACT
            # chain is still filling.
            sched = []
            for ps_i in range(P):
                bl = batches0 if ps_i == 0 else batches_rest
                for bi_i, tb in enumerate(bl):
                    sched.append((ps_i, bi_i, tb, bi_i == len(bl) - 1))
            n0 = len(batches0)
            if P >= 2:
                # move pass-1 batches 0 and 1 up into the pass-0 ramp
                p1b0 = sched.pop(n0)
                p1b1 = sched.pop(n0)
                sched.insert(2, p1b0)
                sched.insert(5, p1b1)

            pend_srep_ps = {ps_i: [] for ps_i in range(P)}
            nclp = max(1, ncl)
            cw = min(KW, 512)
            ramp_i = 0
            for ps, bi, (tb0, tb1), last_b in sched:
                par = ps % 2
                k0 = ps * KW
                pend_srep = pend_srep_ps[ps]

                def flush_srep(fin):
                    tp_, npair_, first_ = pend_srep.pop(0)
                    for i_ in range(npair_):
                        for cl_ in range(nclp):
                            nc.tensor.matmul(
                                srep[(par, cl_)][:],
                                negones[:],
                                tp_[:, i_, cl_ * cw:(cl_ + 1) * cw],
                                start=(first_ and i_ == 0),
                                stop=(fin and i_ == npair_ - 1),
                            )

                if True:
                    n = tb1 - tb0
                    att = atp.tile(
                        [128, n, KW], mm_dtype, name=f"att_{ps}_{bi}",
                        tag="att",
                    )
                    nc.sync.dma_start(
                        att[:], at_d[:, ps, tb0:tb1, :]
                    )
                    # later qt chunks follow the early att batches on
                    # sync so they don't steal HBM bandwidth during the
                    # pipeline ramp; chunk c is triggered one schedule
                    # slot before its lq activation is emitted
                    if ramp_i % 2 == 0 and 3 <= ramp_i // 2 + 2 < len(qsb):
                        qtile, c0_, c1_ = qsb[ramp_i // 2 + 2]
                        nc.sync.dma_start(qtile[:], qt_d[:, c0_:c1_, :])
                    latt = latp.tile(
                        [128, n, KW], mm_dtype,
                        name=f"latt_{ps}_{bi}", tag="latt",
                    )
                    nc.scalar.activation(latt[:], att[:], LN)
                    # trickle in remaining lq activations between the
                    # early batches (their qt DMAs are already in flight)
                    if (
                        qt_dma_emitted < len(qch)
                        and (qt_dma_emitted <= 2
                             or qt_dma_emitted <= ramp_i // 2 + 2)
                    ):
                        qtile, c0_, c1_ = qsb[qt_dma_emitted]
                        nc.scalar.activation(
                            lq[:, c0_:c1_, :], qtile[:], LN
                        )
                        qt_dma_emitted += 1
                    ramp_i += 1
                    tt = ttp.tile(
                        [128, n, KW], mm_dtype, name=f"tt_{ps}_{bi}",
                        tag="tt",
                    )
                    nc.vector.tensor_tensor(tt[:], att[:], latt[:],
                                            op=OP.mult)
                    # before the last batch's cross matmuls, flush all
                    # pending srep matmuls (their pair-sums are long
                    # ready) so only this batch's own self-term work
                    # remains after the final cross matmul
                    if pair and last_b:
                        while pend_srep:
                            flush_srep(False)
                    # cross matmuls
                    for j in range(n):
                        t = tb0 + j
                        for bti in range(2):
                            lhs = lq[:, t, bti * 128:(bti + 1) * 128]
                            for cl in range(nclp):
                                nc.tensor.matmul(
                                    pk[(par, bti, cl)][:],
                                    lhs,
                                    att[:, j, cl * cw:(cl + 1) * cw],
                                    start=(t == 0),
                                    stop=(t == NT - 1),
                                )
                    # self term: pair-sum tt across d-tiles, then matmul.
                    # The srep matmuls for batch bi are emitted after
                    # batch bi+1's cross matmuls (deferred one batch):
                    # if the scalar->DVE chain producing tp lags, the
                    # waiting srep matmul would otherwise head-of-line
                    # block the whole tensor queue.
                    if pair:
                        npair = n // 2
                        tp = tpp.tile(
                            [128, npair, KW], mm_dtype,
                            name=f"tp_{ps}_{bi}", tag="tp",
                        )
                        for i in range(npair):
                            nc.vector.tensor_tensor(
                                tp[:, i, :], tt[:, 2 * i, :],
                                tt[:, 2 * i + 1, :], op=OP.add,
                            )
                        pend_srep.append((tp, npair, bi == 0))

                        if len(pend_srep) > 2:
                            flush_srep(False)
                        if last_b:
                            while len(pend_srep) > 1:
                                flush_srep(False)
                            flush_srep(True)
                    else:
                        for j in range(n):
                            for cl in range(nclp):
                                nc.tensor.matmul(
                                    srep[(par, cl)][:],
                                    negones[:],
                                    tt[:, j, cl * cw:(cl + 1) * cw],
                                    start=(bi == 0 and j == 0),
                                    stop=(last_b and j == n - 1),
                                )

                if ps == 0 and bi == 0:
                    for wi in range(24):
                        nc.tensor.matmul(
                            warm_ps[:], negones[:], negones[:],
                            start=(wi == 0), stop=(wi == 23),
                        )
                if not last_b:
                    continue
                # --- drain pass ps: PSUM -> SBUF -> DRAM -------------
                last_grp = ps == P - 1
                if last_grp:
                    ar_in, koff = ar_b, 0
                elif ps == P - 2:
                    ar_in, koff = ar_c, 0
                else:
                    ar_in, koff = ar_a, ps * KW
                for cl in range(nclp):
                    srep_sb = msbp.tile(
                        [128, cw], F32, name=f"srep_sb_{ps}_{cl}",
                        tag="srep_sb",
                    )
                    nc.vector.tensor_copy(srep_sb[:], srep[(par, cl)][:])
                    for bti in range(2):
                        m_sb = msbp.tile(
                            [128, cw], F32, name=f"m_sb_{ps}_{cl}_{bti}",
                            tag=f"m_sb{bti}",
                        )
                        nc.vector.tensor_tensor(
                            m_sb[:], pk[(par, bti, cl)][:], srep_sb[:],
                            op=OP.add,
                        )
                        nc.gpsimd.dma_start(
                            ar_in[
                                bti * 128:(bti + 1) * 128,
                                koff + cl * cw:koff + (cl + 1) * cw,
                            ],
                            m_sb[:],
                        )
                        m_sb_last = m_sb
                if ps == P - 3:
                    nc.gpsimd.collective_compute(
                        "ReduceScatter",
                        OP.add,
                        replica_groups=[list(range(NCORES))],
                        ins=[ar_a.opt()],
                        outs=[rs_a.opt()],
                    )
                elif ps == P - 2:
                    nc.gpsimd.collective_compute(
                        "ReduceScatter",
                        OP.add,
                        replica_groups=[list(range(NCORES))],
                        ins=[ar_c.opt()],
                        outs=[rs_c.opt()],
                    )
                elif last_grp:
                    nc.gpsimd.collective_compute(
                        "ReduceScatter",
                        OP.add,
                        replica_groups=[list(range(NCORES))],
                        ins=[ar_b.opt()],
                        outs=[rs_b.opt()],
                    )
            # --- tail: msum loads + per-slice epilogues -------------
            # Each msum load is pinned behind the last pass's drain via
            # a tiny WAW copy so the Tile scheduler can never hoist it
            # (or the epilogue that reads it) into the main pipeline --
            # if a ReduceScatter finishes late, nothing upstream stalls.
            # They ride the sync queue, idle once att loads are done.
            msum_a = epp.tile([BS, KWA], F32, bufs=1)
            msum_c = epp.tile([BS, KW], F32, bufs=1)
            msum_b = epp.tile([BS, KW], F32, bufs=1)
            for mt in (msum_a, msum_c, msum_b):
                nc.vector.tensor_copy(mt[0:BS, 0:1], m_sb_last[0:BS, 0:1])
            nc.sync.dma_start(msum_a[:], rs_a[:])
            nc.sync.dma_start(msum_c[:], rs_c[:])
            nc.sync.dma_start(msum_b[:], rs_b[:])

            def emit_epi(col, msum, w, lab_off):
                # argmax partials (value + label) over one k-slice
                nc.vector.tensor_reduce(
                    vcat[:, col:col + 1], msum[:], axis=AX, op=OP.max
                )
                cand = epp.tile(
                    [BS, w], F32, name=f"cand_{col}", tag=f"cand_{col}",
                )
                nc.vector.scalar_tensor_tensor(
                    cand[:], msum[:], vcat[:, col:col + 1],
                    lab1[:, lab_off:lab_off + w],
                    op0=OP.is_equal, op1=OP.mult,
                )
                nc.vector.tensor_reduce(
                    lcat[:, col:col + 1], cand[:], axis=AX, op=OP.max
                )

            emit_epi(0, msum_a, KWA, 0)
            emit_epi(1, msum_c, KW, (P - 2) * KW)
            emit_epi(2, msum_b, KW, (P - 1) * KW)

            # --- final combine across passes ------------------------
            vg = epp.tile([BS, 1], F32, bufs=1)
            nc.vector.tensor_reduce(vg[:], vcat[:], axis=AX, op=OP.max)
            candp = epp.tile([BS, 3], F32, bufs=1)
            nc.vector.scalar_tensor_tensor(
                candp[:], vcat[:], vg[:], lcat[:],
                op0=OP.is_equal, op1=OP.mult,
            )
            lmax = epp.tile([BS, 1], F32, bufs=1)
            nc.vector.tensor_reduce(lmax[:], candp[:], axis=AX, op=OP.max)
            labf = epp.tile([BS, 1], F32, bufs=1)
            nc.vector.tensor_scalar_add(labf[:], lmax[:], -1.0)
            labi = epp.tile([BS, 1], mybir.dt.int32, bufs=1)
            nc.vector.tensor_copy(labi[:], labf[:])
            nc.scalar.dma_start(out_d[:], labi[:])

    nc.compile()
    return nc


def shard_inputs(query, queue_anchor, queue_label, dsh=DSH, d_real=D,
                 passes=None):
    """Host-side layout prep: pad D with 1.0 (log 1 = 0); at in
    pass-major fp16 layout [128, P, NT, KW], qt tile-major
    [128, NT, B]; label row replicated."""
    if passes is None:
        passes = int(os.environ.get("ANCHOR_PASSES", "4"))
    kw = K // passes
    np_dt = np.float16
    q = np.asarray(query, np.float32)
    a = np.asarray(queue_anchor, np.float32)
    lab1 = (np.asarray(queue_label).astype(np.float32) + 1.0)[None, :]
    lab1 = np.ascontiguousarray(np.broadcast_to(lab1, (BS, lab1.shape[1])))
    in_maps = []
    for c in range(NCORES):
        lo = c * dsh
        hi = min((c + 1) * dsh, d_real)
        at = np.ones((dsh, a.shape[0]), np_dt)
        qt = np.ones((dsh, q.shape[0]), np_dt)
        if hi > lo:
            at[: hi - lo, :] = a[:, lo:hi].T.astype(np_dt)
            qt[: hi - lo, :] = q[:, lo:hi].T.astype(np_dt)
        # at: [dsh, K] -> [128, P, NT, KW] (pass-major, contiguous)
        at = np.ascontiguousarray(
            at.reshape(NT, 128, passes, kw).transpose(1, 2, 0, 3)
        )
        # qt: [dsh, B] -> tile-major [128, NT, B]
        qt = np.ascontiguousarray(
            qt.reshape(NT, 128, -1).transpose(1, 0, 2)
        )
        in_maps.append({"at": at, "qt": qt, "lab1": lab1})
    return in_maps


def unshard_out(per_core_outs, split_rs=False):
    """Reassemble the 8 cores' 32-label slices into the [256] output."""
    return np.concatenate([np.asarray(o) for o in per_core_outs])


_NC_CACHE = {}


def _split_rs_active():
    return False


def _get_nc():
    key = (
        os.environ.get("ANCHOR_MM_DTYPE", "float16"),
        int(os.environ.get("ANCHOR_PASSES", "4")),
        int(os.environ.get("ANCHOR_BT", "6")),
        os.environ.get("ANCHOR_PAIR", "1") == "1",
        os.environ.get("ANCHOR_WARM_CC", "1") == "1",
    )
    if key not in _NC_CACHE:
        _NC_CACHE[key] = build(
            mm_dtype=getattr(mybir.dt, key[0]), passes=key[1], bt=key[2],
            pair=key[3], warm_cc=key[4],
        )
    return _NC_CACHE[key]


def kernel(query, queue_anchor, queue_label):
    nc = _get_nc()
    in_maps = shard_inputs(query, queue_anchor, queue_label)
    res = bass_utils.run_bass_kernel_spmd(
        nc, in_maps, core_ids=list(range(NCORES))
    )
    out = unshard_out([res.results[i]["out"] for i in range(NCORES)])
    return out.astype(np.asarray(queue_label).dtype)


# revision 43
# speedup vs baseline: 1.0091x; 1.0091x over previous
"""Trainium2 Bass kernel for KL-divergence 1-NN label lookup (AnchorStore).

reference:
    self[k]  = mean_d a[k,d]*log a[k,d]
    cross    = einsum('kd,bd->kb', a, log q) / D
    kl[b,k]  = self[k] - cross[k,b]
    out[b]   = queue_label[argmin_k kl[b,k]]

Strategy (8 NeuronCores, D-sharded, fp16 operands):
    Each core owns a D-slice (padded with 1.0 so log()=0 contributes
    nothing), shipped as fp16 in d-tile-major layout [128, NT, K].
    Working in SUM units (scale-invariant for argmin):
        m[b,k] = sum_d lq[d,b]*at[d,k] - sum_d at[d,k]*log(at[d,k])
    K is split into P=4 passes of KW columns; passes 0..P-3 share one
    merged ReduceScatter(add) and pass P-2 gets its own, both fully
    overlapped by compute, leaving only a small RS on the tail.
      - TensorE: stationary lq tiles [128d,128b] x moving at [128d,KW]
        accumulate cross into PSUM; the -self term accumulates via a
        (-1)-stationary x pair-summed t = at*log(at) (DVE adds d-tile
        pairs in fp16 to halve the self-matmul column count); those
        srep matmuls are deferred two batches behind their DVE
        producers to avoid tensor-queue head-of-line blocking.
      - ScalarE computes log() (Ln activation) in large batches.
      - Drain: m = pk + srep -> DRAM -> ReduceScatter.
    Pipeline ramp: pass 0 starts with small d-batches and pass 1's
    first two batches are interleaved into the ramp (they reuse the
    same lq tiles, and their PSUM parity banks are free) so the PE has
    a second stream of ready work while DMA/ACT latency fills.
    Tail: msum loads are pinned behind the last drain (WAW copy) so
    the scheduler can never hoist them; per-slice argmax partials
    (value + label via the is_equal trick) run under the last RS, and
    a 3-column combine emits 32 int32 labels per core; host concats.
"""

import os
import sys

import numpy as np

sys.path.insert(0, "/opt/trn_rl_repo")

from concourse import bacc, bass, mybir, tile  # noqa: E402
from concourse import bass_utils  # noqa: E402

K = 2048
B = 256
D = 50257
NCORES = 8
NT = 50             # d-tiles of 128 per core (padded)
DSH = NT * 128      # 6400
BS = B // NCORES    # 32 queries per core after ReduceScatter
F32 = mybir.dt.float32
F16 = mybir.dt.float16


def build(mm_dtype=F16, passes=4, bt=8, pair=True, warm_cc=True):
    """Build the SPMD Bass graph for one core (all cores identical)."""
    P = passes
    KW = K // P              # k columns per pass
    ncl = KW // 512 if KW >= 512 else 0   # full-512 chunks per pass
    assert KW % 512 == 0 or KW in (256,), KW
    nc = bacc.Bacc(
        "TRN2", target_bir_lowering=False, debug=False, num_devices=NCORES
    )
    # pass-major layout: each (pass, tile-range) batch is one fully
    # contiguous region per partition -> max DMA efficiency
    at_d = nc.dram_tensor(
        "at", [128, P, NT, KW], mm_dtype, kind="ExternalInput"
    )
    qt_d = nc.dram_tensor("qt", [128, NT, B], mm_dtype, kind="ExternalInput")
    lab_d = nc.dram_tensor("lab1", [BS, K], F32, kind="ExternalInput")
    out_d = nc.dram_tensor("out", [BS], mybir.dt.int32, kind="ExternalOutput")

    LN = mybir.ActivationFunctionType.Ln
    AX = mybir.AxisListType.X
    OP = mybir.AluOpType

    # d-tile batches (per pass): groups of `bt` tiles, even-sized for
    # pairs.  Pass 0 ramps up with small batches so the matmul pipeline
    # starts early; later passes use full batches (fewer overheads).
    def mk_batches(ramp):
        out = list(ramp)
        t0 = out[-1][1] if out else 0
        while t0 < NT:
            t1 = min(t0 + bt, NT)
            out.append((t0, t1))
            t0 = t1
        return out

    batches0 = mk_batches([(0, 2), (2, 6), (6, 14)])
    batches_rest = mk_batches([])

    # q chunks for lq computation (front chunks small for fast start)
    qch = [(0, 2), (2, 6), (6, 14), (14, 26), (26, 38), (38, NT)]

    with tile.TileContext(nc) as tc:
        with (
            tc.tile_pool(name="const", bufs=1) as constp,
            tc.tile_pool(name="lqp", bufs=1) as lqp,
            tc.tile_pool(name="qinp", bufs=2) as qinp,
            tc.tile_pool(name="atp", bufs=4) as atp,
            tc.tile_pool(name="latp", bufs=4) as latp,
            tc.tile_pool(name="ttp", bufs=4) as ttp,
            tc.tile_pool(name="tpp", bufs=6) as tpp,
            tc.tile_pool(name="msbp", bufs=2) as msbp,
            tc.tile_pool(name="epp", bufs=1) as epp,
            tc.tile_pool(name="psp", bufs=1, space="PSUM") as psp,
            tc.tile_pool(name="dramp", bufs=1, space="DRAM") as dramp,
        ):
            # --- constants / warmup ---------------------------------
            # Tiny dummy DMAs warm each HWDGE/SWDGE queue so the first
            # real input loads don't pay first-transfer setup latency.
            wdma_d = dramp.tile([1, 16], F32, name="wdma_d", bufs=1)
            for eng in (nc.sync, nc.scalar, nc.gpsimd):
                wdma_s = constp.tile([1, 16], F32, name=f"wdma_{eng.engine}")
                eng.dma_start(wdma_s[:], wdma_d[:])


            # --- lq = log(query^T), fp16, resident -------------------
            lq = lqp.tile([128, NT, B], mm_dtype)
            qsb = []
            for ci, (c0_, c1_) in enumerate(qch):
                qtile = qinp.tile(
                    [128, c1_ - c0_, B], mm_dtype, name=f"qtile_{ci}",
                    tag="qtile",
                )
                qsb.append((qtile, c0_, c1_))
            # All qt chunk DMAs are triggered up front on the (idle)
            # gpsimd queue so every chunk is in flight immediately --
            # the ~5us per-DMA completion latency then overlaps instead
            # of serialising with the lq activations.  The tiny dummy
            # Ln pulls the ACT table load forward, under the qt0 DMA.
            dum = constp.tile([128, 16], F32)
            nc.gpsimd.memset(dum[:], 1.0)
            dumo = constp.tile([128, 16], F32)
            nc.scalar.activation(dumo[:], dum[:], LN)
            negones_f = constp.tile([128, 128], F32)
            nc.gpsimd.memset(negones_f[:], -1.0)
            negones = constp.tile([128, 128], mm_dtype)
            nc.vector.tensor_copy(negones[:], negones_f[:])

            # qt triggers go first on gpsimd: nothing else on that
            # queue is needed before ~20us, and the first matmul waits
            # on qt chunk 0 -> lq.
            for qtile, c0_, c1_ in qsb[:3]:
                nc.gpsimd.dma_start(qtile[:], qt_d[:, c0_:c1_, :])
            nc.scalar.activation(
                lq[:, qch[0][0]:qch[0][1], :], qsb[0][0][:], LN
            )

            # ~40 dummy matmuls into a spare PSUM bank while the PE
            # waits for the first lq tile: the HAM clock gate needs
            # ~3.4us of sustained PE activity to lift the 1.2 GHz cold
            # throttle, so the first real matmuls start at full speed.
            warm_ps = psp.tile([128, 128], F32, name="warm_ps")
            for wi in range(40):
                nc.tensor.matmul(
                    warm_ps[:], negones[:], negones[:],
                    start=(wi == 0), stop=(wi == 39),
                )

            if warm_cc:
                # Tiny dummy collective: pre-warms ncfw/credit state on
                # the CC engine and doubles as the cross-core
                # rendezvous long before the first real ReduceScatter.
                w_in = dramp.tile([1, 64], F32)
                w_out = dramp.tile([NCORES, 64], F32)
                w_sb = constp.tile([1, 64], F32)
                nc.gpsimd.memset(w_sb[:], 1.0)
                nc.gpsimd.dma_start(w_in[:], w_sb[:])
                nc.gpsimd.collective_compute(
                    "AllGather",
                    OP.bypass,
                    replica_groups=[list(range(NCORES))],
                    ins=[w_in.opt()],
                    outs=[w_out.opt()],
                )

            lab1 = constp.tile([BS, K], F32)
            nc.gpsimd.dma_start(lab1[:], lab_d[:])

            # --- PSUM accumulators (parity double-buffered) ----------
            pk = {}
            srep = {}
            for par in range(min(2, P)):
                for bti in range(2):
                    for cl in range(max(1, ncl)):
                        pk[(par, bti, cl)] = psp.tile(
                            [128, min(KW, 512)], F32,
                            name=f"pk_{par}_{bti}_{cl}",
                            tag=f"pk_{par}_{bti}_{cl}",
                        )
                for cl in range(max(1, ncl)):
                    srep[(par, cl)] = psp.tile(
                        [128, min(KW, 512)], F32, name=f"srep_{par}_{cl}",
                        tag=f"srep_{par}_{cl}",
                    )

            # per-slice (value, label) partials, combined at the end:
            # slice 0 = merged passes [0, P-2), slice 1 = pass P-2,
            # slice 2 = pass P-1
            vcat = epp.tile([BS, 3], F32, bufs=1)
            lcat = epp.tile([BS, 3], F32, bufs=1)

            qt_dma_emitted = 1  # chunk 0 already emitted

            # Two collective groups: passes [0, P-1) share one big
            # ReduceScatter (launched after pass P-2, fully overlapped
            # by pass P-1 compute); the last pass gets a small RS on
            # the critical tail.  This keeps the CC engine far from
            # saturation and minimises tail latency.
            KWA = (P - 2) * KW
            ar_a = dramp.tile([B, KWA], F32, name="ar_a", bufs=1)
            rs_a = dramp.tile([BS, KWA], F32, name="rs_a", bufs=1)
            ar_c = dramp.tile([B, KW], F32, name="ar_c", bufs=1)
            rs_c = dramp.tile([BS, KW], F32, name="rs_c", bufs=1)
            ar_b = dramp.tile([B, KW], F32, name="ar_b", bufs=1)
            rs_b = dramp.tile([BS, KW], F32, name="rs_b", bufs=1)

            # Explicit (ps, bi) schedule: pass-1's first two batches
            # are interleaved into pass-0's ramp (they share lq tiles
            # and par=1 PSUM banks are free), so the tensor engine has
            # a second stream of ready work while pass-0's DMA/ACT
            # chain is still filling.
            sched = []
            for ps_i in range(P):
                bl = batches0 if ps_i == 0 else batches_rest
                for bi_i, tb in enumerate(bl):
                    sched.append((ps_i, bi_i, tb, bi_i == len(bl) - 1))
            n0 = len(batches0)
            if P >= 2:
                # move pass-1 batches 0 and 1 up into the pass-0 ramp
                p1b0 = sched.pop(n0)
                p1b1 = sched.pop(n0)
                sched.insert(2, p1b0)
                sched.insert(5, p1b1)

            pend_srep_ps = {ps_i: [] for ps_i in range(P)}
            nclp = max(1, ncl)
            cw = min(KW, 512)
            ramp_i = 0
            for ps, bi, (tb0, tb1), last_b in sched:
                par = ps % 2
                k0 = ps * KW
                pend_srep = pend_srep_ps[ps]

                def flush_srep(fin):
                    tp_, npair_, first_ = pend_srep.pop(0)
                    for i_ in range(npair_):
                        for cl_ in range(nclp):
                            nc.tensor.matmul(
                                srep[(par, cl_)][:],
                                negones[:],
                                tp_[:, i_, cl_ * cw:(cl_ + 1) * cw],
                                start=(first_ and i_ == 0),
                                stop=(fin and i_ == npair_ - 1),
                            )

                if True:
                    n = tb1 - tb0
                    att = atp.tile(
                        [128, n, KW], mm_dtype, name=f"att_{ps}_{bi}",
                        tag="att",
                    )
                    nc.sync.dma_start(
                        att[:], at_d[:, ps, tb0:tb1, :]
                    )
                    # later qt chunks follow the early att batches on
                    # sync so they don't steal HBM bandwidth during the
                    # pipeline ramp; chunk c is triggered one schedule
                    # slot before its lq activation is emitted
                    if ramp_i % 2 == 0 and 3 <= ramp_i // 2 + 2 < len(qsb):
                        qtile, c0_, c1_ = qsb[ramp_i // 2 + 2]
                        nc.sync.dma_start(qtile[:], qt_d[:, c0_:c1_, :])
                    latt = latp.tile(
                        [128, n, KW], mm_dtype,
                        name=f"latt_{ps}_{bi}", tag="latt",
                    )
                    nc.scalar.activation(latt[:], att[:], LN)
                    # trickle in remaining lq activations between the
                    # early batches (their qt DMAs are already in flight)
                    if (
                        qt_dma_emitted < len(qch)
                        and (qt_dma_emitted <= 2
                             or qt_dma_emitted <= ramp_i // 2 + 2)
                    ):
                        qtile, c0_, c1_ = qsb[qt_dma_emitted]
                        nc.scalar.activation(
                            lq[:, c0_:c1_, :], qtile[:], LN
                        )
                        qt_dma_emitted += 1
                    ramp_i += 1
                    tt = ttp.tile(
                        [128, n, KW], mm_dtype, name=f"tt_{ps}_{bi}",
                        tag="tt",
                    )
                    nc.vector.tensor_tensor(tt[:], att[:], latt[:],
                                            op=OP.mult)
                    # before the last batch's cross matmuls, flush all
                    # pending srep matmuls (their pair-sums are long
                    # ready) so only this batch's own self-term work
                    # remains after the final cross matmul
                    if pair and last_b:
                        while pend_srep:
                            flush_srep(False)
                    # cross matmuls
                    for j in range(n):
                        t = tb0 + j
                        for bti in range(2):
                            lhs = lq[:, t, bti * 128:(bti + 1) * 128]
                            for cl in range(nclp):
                                nc.tensor.matmul(
                                    pk[(par, bti, cl)][:],
                                    lhs,
                                    att[:, j, cl * cw:(cl + 1) * cw],
                                    start=(t == 0),
                                    stop=(t == NT - 1),
                                )
                    # self term: pair-sum tt across d-tiles, then matmul.
                    # The srep matmuls for batch bi are emitted after
                    # batch bi+1's cross matmuls (deferred one batch):
                    # if the scalar->DVE chain producing tp lags, the
                    # waiting srep matmul would otherwise head-of-line
                    # block the whole tensor queue.
                    if pair:
                        npair = n // 2
                        tp = tpp.tile(
                            [128, npair, KW], mm_dtype,
                            name=f"tp_{ps}_{bi}", tag="tp",
                        )
                        for i in range(npair):
                            nc.vector.tensor_tensor(
                                tp[:, i, :], tt[:, 2 * i, :],
                                tt[:, 2 * i + 1, :], op=OP.add,
                            )
                        pend_srep.append((tp, npair, bi == 0))

                        if len(pend_srep) > 2:
                            flush_srep(False)
                        if last_b:
                            while len(pend_srep) > 1:
                                flush_srep(False)
                            flush_srep(True)
                    else:
                        for j in range(n):
                            for cl in range(nclp):
                                nc.tensor.matmul(
                                    srep[(par, cl)][:],
                                    negones[:],
                                    tt[:, j, cl * cw:(cl + 1) * cw],
                                    start=(bi == 0 and j == 0),
                                    stop=(last_b and j == n - 1),
                                )

                if ps == 0 and bi == 0:
                    for wi in range(24):
                        nc.tensor.matmul(
                            warm_ps[:], negones[:], negones[:],
                            start=(wi == 0), stop=(wi == 23),
                        )
                if not last_b:
                    continue
                # --- drain pass ps: PSUM -> SBUF -> DRAM -------------
                last_grp = ps == P - 1
                if last_grp:
                    ar_in, koff = ar_b, 0
                elif ps == P - 2:
                    ar_in, koff = ar_c, 0
                else:
                    ar_in, koff = ar_a, ps * KW
                for cl in range(nclp):
                    srep_sb = msbp.tile(
                        [128, cw], F32, name=f"srep_sb_{ps}_{cl}",
                        tag="srep_sb",
                    )
                    nc.vector.tensor_copy(srep_sb[:], srep[(par, cl)][:])
                    for bti in range(2):
                        m_sb = msbp.tile(
                            [128, cw], F32, name=f"m_sb_{ps}_{cl}_{bti}",
                            tag=f"m_sb{bti}",
                        )
                        nc.vector.tensor_tensor(
                            m_sb[:], pk[(par, bti, cl)][:], srep_sb[:],
                            op=OP.add,
                        )
                        nc.gpsimd.dma_start(
                            ar_in[
                                bti * 128:(bti + 1) * 128,
                                koff + cl * cw:koff + (cl + 1) * cw,
                            ],
                            m_sb[:],
                        )
                        m_sb_last = m_sb
                if ps == P - 3:
                    nc.gpsimd.collective_compute(
                        "ReduceScatter",
                        OP.add,
                        replica_groups=[list(range(NCORES))],
                        ins=[ar_a.opt()],
                        outs=[rs_a.opt()],
                    )
                elif ps == P - 2:
                    nc.gpsimd.collective_compute(
                        "ReduceScatter",
                        OP.add,
                        replica_groups=[list(range(NCORES))],
                        ins=[ar_c.opt()],
                        outs=[rs_c.opt()],
                    )
                elif last_grp:
                    nc.gpsimd.collective_compute(
                        "ReduceScatter",
                        OP.add,
                        replica_groups=[list(range(NCORES))],
                        ins=[ar_b.opt()],
                        outs=[rs_b.opt()],
                    )
            # --- tail: msum loads + per-slice epilogues -------------
            # Each msum load is pinned behind the last pass's drain via
            # a tiny WAW copy so the Tile scheduler can never hoist it
            # (or the epilogue that reads it) into the main pipeline --
            # if a ReduceScatter finishes late, nothing upstream stalls.
            # They ride the sync queue, idle once att loads are done.
            msum_a = epp.tile([BS, KWA], F32, bufs=1)
            msum_c = epp.tile([BS, KW], F32, bufs=1)
            msum_b = epp.tile([BS, KW], F32, bufs=1)
            for mt in (msum_a, msum_c, msum_b):
                nc.vector.tensor_copy(mt[0:BS, 0:1], m_sb_last[0:BS, 0:1])
            nc.sync.dma_start(msum_a[:], rs_a[:])
            nc.sync.dma_start(msum_c[:], rs_c[:])
            nc.sync.dma_start(msum_b[:], rs_b[:])

            def emit_epi(col, msum, w, lab_off):
                # argmax partials (value + label) over one k-slice
                nc.vector.tensor_reduce(
                    vcat[:, col:col + 1], msum[:], axis=AX, op=OP.max
                )
                cand = epp.tile(
                    [BS, w], F32, name=f"cand_{col}", tag=f"cand_{col}",
                )
                nc.vector.scalar_tensor_tensor(
                    cand[:], msum[:], vcat[:, col:col + 1],
                    lab1[:, lab_off:lab_off + w],
                    op0=OP.is_equal, op1=OP.mult,
                )
                nc.vector.tensor_reduce(
                    lcat[:, col:col + 1], cand[:], axis=AX, op=OP.max
                )

            emit_epi(0, msum_a, KWA, 0)
            emit_epi(1, msum_c, KW, (P - 2) * KW)
            emit_epi(2, msum_b, KW, (P - 1) * KW)

            # --- final combine across passes ------------------------
            vg = epp.tile([BS, 1], F32, bufs=1)
            nc.vector.tensor_reduce(vg[:], vcat[:], axis=AX, op=OP.max)
            candp = epp.tile([BS, 3], F32, bufs=1)
            nc.vector.scalar_tensor_tensor(
                candp[:], vcat[:], vg[:], lcat[:],
                op0=OP.is_equal, op1=OP.mult,
            )
            lmax = epp.tile([BS, 1], F32, bufs=1)
            nc.vector.tensor_reduce(lmax[:], candp[:], axis=AX, op=OP.max)
            labf = epp.tile([BS, 1], F32, bufs=1)
            nc.vector.tensor_scalar_add(labf[:], lmax[:], -1.0)
            labi = epp.tile([BS, 1], mybir.dt.int32, bufs=1)
            nc.vector.tensor_copy(labi[:], labf[:])
            nc.scalar.dma_start(out_d[:], labi[:])

    nc.compile()
    return nc


def shard_inputs(query, queue_anchor, queue_label, dsh=DSH, d_real=D,
                 passes=None):
    """Host-side layout prep: pad D with 1.0 (log 1 = 0); at in
    pass-major fp16 layout [128, P, NT, KW], qt tile-major
    [128, NT, B]; label row replicated."""
    if passes is None:
        passes = int(os.environ.get("ANCHOR_PASSES", "4"))
    kw = K // passes
    np_dt = np.float16
    q = np.asarray(query, np.float32)
    a = np.asarray(queue_anchor, np.float32)
    lab1 = (np.asarray(queue_label).astype(np.float32) + 1.0)[None, :]
    lab1 = np.ascontiguousarray(np.broadcast_to(lab1, (BS, lab1.shape[1])))
    in_maps = []
    for c in range(NCORES):
        lo = c * dsh
        hi = min((c + 1) * dsh, d_real)
        at = np.ones((dsh, a.shape[0]), np_dt)
        qt = np.ones((dsh, q.shape[0]), np_dt)
        if hi > lo:
            at[: hi - lo, :] = a[:, lo:hi].T.astype(np_dt)
            qt[: hi - lo, :] = q[:, lo:hi].T.astype(np_dt)
        # at: [dsh, K] -> [128, P, NT, KW] (pass-major, contiguous)
        at = np.ascontiguousarray(
            at.reshape(NT, 128, passes, kw).transpose(1, 2, 0, 3)
        )
        # qt: [dsh, B] -> tile-major [128, NT, B]
        qt = np.ascontiguousarray(
            qt.reshape(NT, 128, -1).transpose(1, 0, 2)
        )
        in_maps.append({"at": at, "qt": qt, "lab1": lab1})
    return in_maps


def unshard_out(per_core_outs, split_rs=False):
    """Reassemble the 8 cores' 32-label slices into the [256] output."""
    return np.concatenate([np.asarray(o) for o in per_core_outs])


_NC_CACHE = {}


def _split_rs_active():
    return False


def _get_nc():
    key = (
        os.environ.get("ANCHOR_MM_DTYPE", "float16"),
        int(os.environ.get("ANCHOR_PASSES", "4")),
        int(os.environ.get("ANCHOR_BT", "6")),
        os.environ.get("ANCHOR_PAIR", "1") == "1",
        os.environ.get("ANCHOR_WARM_CC", "1") == "1",
    )
    if key not in _NC_CACHE:
        _NC_CACHE[key] = build(
            mm_dtype=getattr(mybir.dt, key[0]), passes=key[1], bt=key[2],
            pair=key[3], warm_cc=key[4],
        )
    return _NC_CACHE[key]


def kernel(query, queue_anchor, queue_label):
    nc = _get_nc()
    in_maps = shard_inputs(query, queue_anchor, queue_label)
    res = bass_utils.run_bass_kernel_spmd(
        nc, in_maps, core_ids=list(range(NCORES))
    )
    out = unshard_out([res.results[i]["out"] for i in range(NCORES)])
    return out.astype(np.asarray(queue_label).dtype)


# revision 45
# speedup vs baseline: 1.0132x; 1.0040x over previous
"""Trainium2 Bass kernel for KL-divergence 1-NN label lookup (AnchorStore).

reference:
    self[k]  = mean_d a[k,d]*log a[k,d]
    cross    = einsum('kd,bd->kb', a, log q) / D
    kl[b,k]  = self[k] - cross[k,b]
    out[b]   = queue_label[argmin_k kl[b,k]]

Strategy (8 NeuronCores, D-sharded, fp16 operands):
    Each core owns a D-slice (padded with 1.0 so log()=0 contributes
    nothing), shipped as fp16 in d-tile-major layout [128, NT, K].
    Working in SUM units (scale-invariant for argmin):
        m[b,k] = sum_d lq[d,b]*at[d,k] - sum_d at[d,k]*log(at[d,k])
    K is split into P=4 passes of KW columns; passes 0..P-3 share one
    merged ReduceScatter(add) and pass P-2 gets its own, both fully
    overlapped by compute, leaving only a small RS on the tail.
      - TensorE: stationary lq tiles [128d,128b] x moving at [128d,KW]
        accumulate cross into PSUM; the -self term accumulates via a
        (-1)-stationary x pair-summed t = at*log(at) (DVE adds d-tile
        pairs in fp16 to halve the self-matmul column count); those
        srep matmuls are deferred two batches behind their DVE
        producers to avoid tensor-queue head-of-line blocking.
      - ScalarE computes log() (Ln activation) in large batches.
      - Drain: m = pk + srep -> DRAM -> ReduceScatter.
    Pipeline ramp: pass 0 starts with small d-batches and pass 1's
    first two batches are interleaved into the ramp (they reuse the
    same lq tiles, and their PSUM parity banks are free) so the PE has
    a second stream of ready work while DMA/ACT latency fills.
    Tail: msum loads are pinned behind the last drain (WAW copy) so
    the scheduler can never hoist them; per-slice argmax partials
    (value + label via the is_equal trick) run under the last RS, and
    a 3-column combine emits 32 int32 labels per core; host concats.
"""

import os
import sys

import numpy as np

sys.path.insert(0, "/opt/trn_rl_repo")

from concourse import bacc, bass, mybir, tile  # noqa: E402
from concourse import bass_utils  # noqa: E402

K = 2048
B = 256
D = 50257
NCORES = 8
NT = 50             # d-tiles of 128 per core (padded)
DSH = NT * 128      # 6400
BS = B // NCORES    # 32 queries per core after ReduceScatter
F32 = mybir.dt.float32
F16 = mybir.dt.float16


def build(mm_dtype=F16, passes=4, bt=8, pair=True, warm_cc=True):
    """Build the SPMD Bass graph for one core (all cores identical)."""
    P = passes
    KW = K // P              # k columns per pass
    ncl = KW // 512 if KW >= 512 else 0   # full-512 chunks per pass
    assert KW % 512 == 0 or KW in (256,), KW
    nc = bacc.Bacc(
        "TRN2", target_bir_lowering=False, debug=False, num_devices=NCORES
    )
    # pass-major layout: each (pass, tile-range) batch is one fully
    # contiguous region per partition -> max DMA efficiency
    at_d = nc.dram_tensor(
        "at", [128, P, NT, KW], mm_dtype, kind="ExternalInput"
    )
    qt_d = nc.dram_tensor("qt", [128, NT, B], mm_dtype, kind="ExternalInput")
    lab_d = nc.dram_tensor("lab1", [BS, K], F32, kind="ExternalInput")
    out_d = nc.dram_tensor("out", [BS], mybir.dt.int32, kind="ExternalOutput")

    LN = mybir.ActivationFunctionType.Ln
    AX = mybir.AxisListType.X
    OP = mybir.AluOpType

    # d-tile batches (per pass): groups of `bt` tiles, even-sized for
    # pairs.  Pass 0 ramps up with small batches so the matmul pipeline
    # starts early; later passes use full batches (fewer overheads).
    def mk_batches(ramp):
        out = list(ramp)
        t0 = out[-1][1] if out else 0
        while t0 < NT:
            t1 = min(t0 + bt, NT)
            out.append((t0, t1))
            t0 = t1
        return out

    batches0 = mk_batches([(0, 2), (2, 6), (6, 14)])
    batches_rest = mk_batches([])

    # q chunks for lq computation (front chunks small for fast start)
    qch = [(0, 2), (2, 6), (6, 14), (14, 26), (26, 38), (38, NT)]

    with tile.TileContext(nc) as tc:
        with (
            tc.tile_pool(name="const", bufs=1) as constp,
            tc.tile_pool(name="lqp", bufs=1) as lqp,
            tc.tile_pool(name="qinp", bufs=2) as qinp,
            tc.tile_pool(name="atp", bufs=4) as atp,
            tc.tile_pool(name="latp", bufs=4) as latp,
            tc.tile_pool(name="ttp", bufs=4) as ttp,
            tc.tile_pool(name="tpp", bufs=6) as tpp,
            tc.tile_pool(name="msbp", bufs=2) as msbp,
            tc.tile_pool(name="epp", bufs=1) as epp,
            tc.tile_pool(name="psp", bufs=1, space="PSUM") as psp,
            tc.tile_pool(name="dramp", bufs=1, space="DRAM") as dramp,
        ):
            # --- constants / warmup ---------------------------------
            # Tiny dummy DMAs warm each HWDGE/SWDGE queue so the first
            # real input loads don't pay first-transfer setup latency.
            wdma_d = dramp.tile([1, 16], F32, name="wdma_d", bufs=1)
            for eng in (nc.sync, nc.scalar, nc.gpsimd):
                wdma_s = constp.tile([1, 16], F32, name=f"wdma_{eng.engine}")
                eng.dma_start(wdma_s[:], wdma_d[:])


            # --- lq = log(query^T), fp16, resident -------------------
            lq = lqp.tile([128, NT, B], mm_dtype)
            qsb = []
            for ci, (c0_, c1_) in enumerate(qch):
                qtile = qinp.tile(
                    [128, c1_ - c0_, B], mm_dtype, name=f"qtile_{ci}",
                    tag="qtile",
                )
                qsb.append((qtile, c0_, c1_))
            # All qt chunk DMAs are triggered up front on the (idle)
            # gpsimd queue so every chunk is in flight immediately --
            # the ~5us per-DMA completion latency then overlaps instead
            # of serialising with the lq activations.  The tiny dummy
            # Ln pulls the ACT table load forward, under the qt0 DMA.
            dum = constp.tile([128, 16], F32)
            nc.gpsimd.memset(dum[:], 1.0)
            dumo = constp.tile([128, 16], F32)
            nc.scalar.activation(dumo[:], dum[:], LN)
            negones_f = constp.tile([128, 128], F32)
            nc.gpsimd.memset(negones_f[:], -1.0)
            negones = constp.tile([128, 128], mm_dtype)
            nc.vector.tensor_copy(negones[:], negones_f[:])

            # qt triggers go first on gpsimd: nothing else on that
            # queue is needed before ~20us, and the first matmul waits
            # on qt chunk 0 -> lq.
            for qtile, c0_, c1_ in qsb[:3]:
                nc.gpsimd.dma_start(qtile[:], qt_d[:, c0_:c1_, :])
            nc.scalar.activation(
                lq[:, qch[0][0]:qch[0][1], :], qsb[0][0][:], LN
            )

            # ~40 dummy matmuls into a spare PSUM bank while the PE
            # waits for the first lq tile: the HAM clock gate needs
            # ~3.4us of sustained PE activity to lift the 1.2 GHz cold
            # throttle, so the first real matmuls start at full speed.
            warm_ps = psp.tile([128, 128], F32, name="warm_ps")
            for wi in range(40):
                nc.tensor.matmul(
                    warm_ps[:], negones[:], negones[:],
                    start=(wi == 0), stop=(wi == 39),
                )

            if warm_cc:
                # Tiny dummy collective: pre-warms ncfw/credit state on
                # the CC engine and doubles as the cross-core
                # rendezvous long before the first real ReduceScatter.
                w_in = dramp.tile([1, 64], F32)
                w_out = dramp.tile([NCORES, 64], F32)
                w_sb = constp.tile([1, 64], F32)
                nc.gpsimd.memset(w_sb[:], 1.0)
                nc.gpsimd.dma_start(w_in[:], w_sb[:])
                nc.gpsimd.collective_compute(
                    "AllGather",
                    OP.bypass,
                    replica_groups=[list(range(NCORES))],
                    ins=[w_in.opt()],
                    outs=[w_out.opt()],
                )

            lab1 = constp.tile([BS, K], F32)
            nc.gpsimd.dma_start(lab1[:], lab_d[:])

            # --- PSUM accumulators (parity double-buffered) ----------
            pk = {}
            srep = {}
            for par in range(min(2, P)):
                for bti in range(2):
                    for cl in range(max(1, ncl)):
                        pk[(par, bti, cl)] = psp.tile(
                            [128, min(KW, 512)], F32,
                            name=f"pk_{par}_{bti}_{cl}",
                            tag=f"pk_{par}_{bti}_{cl}",
                        )
                for cl in range(max(1, ncl)):
                    srep[(par, cl)] = psp.tile(
                        [128, min(KW, 512)], F32, name=f"srep_{par}_{cl}",
                        tag=f"srep_{par}_{cl}",
                    )

            # per-slice (value, label) partials, combined at the end:
            # slice 0 = merged passes [0, P-2), slice 1 = pass P-2,
            # slice 2 = pass P-1
            vcat = epp.tile([BS, 3], F32, bufs=1)
            lcat = epp.tile([BS, 3], F32, bufs=1)

            qt_dma_emitted = 1  # chunk 0 already emitted

            # Two collective groups: passes [0, P-1) share one big
            # ReduceScatter (launched after pass P-2, fully overlapped
            # by pass P-1 compute); the last pass gets a small RS on
            # the critical tail.  This keeps the CC engine far from
            # saturation and minimises tail latency.
            KWA = (P - 2) * KW
            ar_a = dramp.tile([B, KWA], F32, name="ar_a", bufs=1)
            rs_a = dramp.tile([BS, KWA], F32, name="rs_a", bufs=1)
            ar_c = dramp.tile([B, KW], F32, name="ar_c", bufs=1)
            rs_c = dramp.tile([BS, KW], F32, name="rs_c", bufs=1)
            ar_b = dramp.tile([B, KW], F32, name="ar_b", bufs=1)
            rs_b = dramp.tile([BS, KW], F32, name="rs_b", bufs=1)

            # Explicit (ps, bi) schedule: pass-1's first two batches
            # are interleaved into pass-0's ramp (they share lq tiles
            # and par=1 PSUM banks are free), so the tensor engine has
            # a second stream of ready work while pass-0's DMA/ACT
            # chain is still filling.
            sched = []
            for ps_i in range(P):
                bl = batches0 if ps_i == 0 else batches_rest
                for bi_i, tb in enumerate(bl):
                    sched.append((ps_i, bi_i, tb, bi_i == len(bl) - 1))
            n0 = len(batches0)
            if P >= 2:
                # move pass-1 batches 0 and 1 up into the pass-0 ramp
                p1b0 = sched.pop(n0)
                p1b1 = sched.pop(n0)
                sched.insert(2, p1b0)
                sched.insert(5, p1b1)

            pend_srep_ps = {ps_i: [] for ps_i in range(P)}
            nclp = max(1, ncl)
            cw = min(KW, 512)
            ramp_i = 0
            for ps, bi, (tb0, tb1), last_b in sched:
                par = ps % 2
                k0 = ps * KW
                pend_srep = pend_srep_ps[ps]

                def flush_srep(fin):
                    tp_, npair_, first_ = pend_srep.pop(0)
                    for i_ in range(npair_):
                        for cl_ in range(nclp):
                            nc.tensor.matmul(
                                srep[(par, cl_)][:],
                                negones[:],
                                tp_[:, i_, cl_ * cw:(cl_ + 1) * cw],
                                start=(first_ and i_ == 0),
                                stop=(fin and i_ == npair_ - 1),
                            )

                if True:
                    n = tb1 - tb0
                    att = atp.tile(
                        [128, n, KW], mm_dtype, name=f"att_{ps}_{bi}",
                        tag="att",
                    )
                    nc.sync.dma_start(
                        att[:], at_d[:, ps, tb0:tb1, :]
                    )
                    # later qt chunks follow the early att batches on
                    # sync so they don't steal HBM bandwidth during the
                    # pipeline ramp; chunk c is triggered one schedule
                    # slot before its lq activation is emitted
                    if ramp_i % 2 == 0 and 3 <= ramp_i // 2 + 2 < len(qsb):
                        qtile, c0_, c1_ = qsb[ramp_i // 2 + 2]
                        nc.sync.dma_start(qtile[:], qt_d[:, c0_:c1_, :])
                    latt = latp.tile(
                        [128, n, KW], mm_dtype,
                        name=f"latt_{ps}_{bi}", tag="latt",
                    )
                    nc.scalar.activation(latt[:], att[:], LN)
                    # trickle in remaining lq activations between the
                    # early batches (their qt DMAs are already in flight)
                    if (
                        qt_dma_emitted < len(qch)
                        and (qt_dma_emitted <= 2
                             or qt_dma_emitted <= ramp_i // 2 + 2)
                    ):
                        qtile, c0_, c1_ = qsb[qt_dma_emitted]
                        nc.scalar.activation(
                            lq[:, c0_:c1_, :], qtile[:], LN
                        )
                        qt_dma_emitted += 1
                    ramp_i += 1
                    tt = ttp.tile(
                        [128, n, KW], mm_dtype, name=f"tt_{ps}_{bi}",
                        tag="tt",
                    )
                    nc.vector.tensor_tensor(tt[:], att[:], latt[:],
                                            op=OP.mult)
                    # before the last batch's cross matmuls, flush all
                    # pending srep matmuls (their pair-sums are long
                    # ready) so only this batch's own self-term work
                    # remains after the final cross matmul
                    if pair and last_b:
                        while pend_srep:
                            flush_srep(False)
                    # cross matmuls
                    for j in range(n):
                        t = tb0 + j
                        for bti in range(2):
                            lhs = lq[:, t, bti * 128:(bti + 1) * 128]
                            for cl in range(nclp):
                                nc.tensor.matmul(
                                    pk[(par, bti, cl)][:],
                                    lhs,
                                    att[:, j, cl * cw:(cl + 1) * cw],
                                    start=(t == 0),
                                    stop=(t == NT - 1),
                                )
                    # self term: pair-sum tt across d-tiles, then matmul.
                    # The srep matmuls for batch bi are emitted after
                    # batch bi+1's cross matmuls (deferred one batch):
                    # if the scalar->DVE chain producing tp lags, the
                    # waiting srep matmul would otherwise head-of-line
                    # block the whole tensor queue.
                    if pair:
                        npair = n // 2
                        tp = tpp.tile(
                            [128, npair, KW], mm_dtype,
                            name=f"tp_{ps}_{bi}", tag="tp",
                        )
                        for i in range(npair):
                            nc.vector.tensor_tensor(
                                tp[:, i, :], tt[:, 2 * i, :],
                                tt[:, 2 * i + 1, :], op=OP.add,
                            )
                        pend_srep.append((tp, npair, bi == 0))

                        if len(pend_srep) > 2:
                            flush_srep(False)
                        if last_b:
                            while len(pend_srep) > 1:
                                flush_srep(False)
                            flush_srep(True)
                    else:
                        for j in range(n):
                            for cl in range(nclp):
                                nc.tensor.matmul(
                                    srep[(par, cl)][:],
                                    negones[:],
                                    tt[:, j, cl * cw:(cl + 1) * cw],
                                    start=(bi == 0 and j == 0),
                                    stop=(last_b and j == n - 1),
                                )

                if ps == 0 and bi == 0:
                    for wi in range(24):
                        nc.tensor.matmul(
                            warm_ps[:], negones[:], negones[:],
                            start=(wi == 0), stop=(wi == 23),
                        )
                if not last_b:
                    continue
                # --- drain pass ps: PSUM -> SBUF -> DRAM -------------
                last_grp = ps == P - 1
                if last_grp:
                    ar_in, koff = ar_b, 0
                elif ps == P - 2:
                    ar_in, koff = ar_c, 0
                else:
                    ar_in, koff = ar_a, ps * KW
                for cl in range(nclp):
                    srep_sb = msbp.tile(
                        [128, cw], F32, name=f"srep_sb_{ps}_{cl}",
                        tag="srep_sb",
                    )
                    nc.vector.tensor_copy(srep_sb[:], srep[(par, cl)][:])
                    for bti in range(2):
                        m_sb = msbp.tile(
                            [128, cw], F32, name=f"m_sb_{ps}_{cl}_{bti}",
                            tag=f"m_sb{bti}",
                        )
                        nc.vector.tensor_tensor(
                            m_sb[:], pk[(par, bti, cl)][:], srep_sb[:],
                            op=OP.add,
                        )
                        nc.gpsimd.dma_start(
                            ar_in[
                                bti * 128:(bti + 1) * 128,
                                koff + cl * cw:koff + (cl + 1) * cw,
                            ],
                            m_sb[:],
                        )
                        m_sb_last = m_sb
                if ps == P - 3:
                    nc.gpsimd.collective_compute(
                        "ReduceScatter",
                        OP.add,
                        replica_groups=[list(range(NCORES))],
                        ins=[ar_a.opt()],
                        outs=[rs_a.opt()],
                    )
                elif ps == P - 2:
                    nc.gpsimd.collective_compute(
                        "ReduceScatter",
                        OP.add,
                        replica_groups=[list(range(NCORES))],
                        ins=[ar_c.opt()],
                        outs=[rs_c.opt()],
                    )
                elif last_grp:
                    nc.gpsimd.collective_compute(
                        "ReduceScatter",
                        OP.add,
                        replica_groups=[list(range(NCORES))],
                        ins=[ar_b.opt()],
                        outs=[rs_b.opt()],
                    )
            # --- tail: msum loads + per-slice epilogues -------------
            # Each msum load is pinned behind the last pass's drain via
            # a tiny WAW copy so the Tile scheduler can never hoist it
            # (or the epilogue that reads it) into the main pipeline --
            # if a ReduceScatter finishes late, nothing upstream stalls.
            # They ride the sync queue, idle once att loads are done.
            msum_a = epp.tile([BS, KWA], F32, bufs=1)
            msum_c = epp.tile([BS, KW], F32, bufs=1)
            msum_b = epp.tile([BS, KW], F32, bufs=1)
            for mt in (msum_a, msum_c, msum_b):
                nc.vector.tensor_copy(mt[0:BS, 0:1], m_sb_last[0:BS, 0:1])
            nc.sync.dma_start(msum_a[:], rs_a[:])
            nc.sync.dma_start(msum_c[:], rs_c[:])
            nc.sync.dma_start(msum_b[:], rs_b[:])

            def emit_epi(col, msum, w, lab_off):
                # argmax partials (value + label) over one k-slice
                nc.vector.tensor_reduce(
                    vcat[:, col:col + 1], msum[:], axis=AX, op=OP.max
                )
                cand = epp.tile(
                    [BS, w], F32, name=f"cand_{col}", tag=f"cand_{col}",
                )
                nc.vector.scalar_tensor_tensor(
                    cand[:], msum[:], vcat[:, col:col + 1],
                    lab1[:, lab_off:lab_off + w],
                    op0=OP.is_equal, op1=OP.mult,
                )
                nc.vector.tensor_reduce(
                    lcat[:, col:col + 1], cand[:], axis=AX, op=OP.max
                )

            emit_epi(0, msum_a, KWA, 0)
            emit_epi(1, msum_c, KW, (P - 2) * KW)
            emit_epi(2, msum_b, KW, (P - 1) * KW)

            # --- final combine across passes ------------------------
            vg = epp.tile([BS, 1], F32, bufs=1)
            nc.vector.tensor_reduce(vg[:], vcat[:], axis=AX, op=OP.max)
            candp = epp.tile([BS, 3], F32, bufs=1)
            nc.vector.scalar_tensor_tensor(
                candp[:], vcat[:], vg[:], lcat[:],
                op0=OP.is_equal, op1=OP.mult,
            )
            lmax = epp.tile([BS, 1], F32, bufs=1)
            nc.vector.tensor_reduce(lmax[:], candp[:], axis=AX, op=OP.max)
            labf = epp.tile([BS, 1], F32, bufs=1)
            nc.vector.tensor_scalar_add(labf[:], lmax[:], -1.0)
            labi = epp.tile([BS, 1], mybir.dt.int32, bufs=1)
            nc.vector.tensor_copy(labi[:], labf[:])
            nc.scalar.dma_start(out_d[:], labi[:])

    nc.compile()
    return nc


def shard_inputs(query, queue_anchor, queue_label, dsh=DSH, d_real=D,
                 passes=None):
    """Host-side layout prep: pad D with 1.0 (log 1 = 0); at in
    pass-major fp16 layout [128, P, NT, KW], qt tile-major
    [128, NT, B]; label row replicated."""
    if passes is None:
        passes = int(os.environ.get("ANCHOR_PASSES", "4"))
    kw = K // passes
    np_dt = np.float16
    q = np.asarray(query, np.float32)
    a = np.asarray(queue_anchor, np.float32)
    lab1 = (np.asarray(queue_label).astype(np.float32) + 1.0)[None, :]
    lab1 = np.ascontiguousarray(np.broadcast_to(lab1, (BS, lab1.shape[1])))
    in_maps = []
    for c in range(NCORES):
        lo = c * dsh
        hi = min((c + 1) * dsh, d_real)
        at = np.ones((dsh, a.shape[0]), np_dt)
        qt = np.ones((dsh, q.shape[0]), np_dt)
        if hi > lo:
            at[: hi - lo, :] = a[:, lo:hi].T.astype(np_dt)
            qt[: hi - lo, :] = q[:, lo:hi].T.astype(np_dt)
        # at: [dsh, K] -> [128, P, NT, KW] (pass-major, contiguous)
        at = np.ascontiguousarray(
            at.reshape(NT, 128, passes, kw).transpose(1, 2, 0, 3)
        )
        # qt: [dsh, B] -> tile-major [128, NT, B]
        qt = np.ascontiguousarray(
            qt.reshape(NT, 128, -1).transpose(1, 0, 2)
        )
        in_maps.append({"at": at, "qt": qt, "lab1": lab1})
    return in_maps


def unshard_out(per_core_outs, split_rs=False):
    """Reassemble the 8 cores' 32-label slices into the [256] output."""
    return np.concatenate([np.asarray(o) for o in per_core_outs])


_NC_CACHE = {}


def _split_rs_active():
    return False


def _get_nc():
    key = (
        os.environ.get("ANCHOR_MM_DTYPE", "float16"),
        int(os.environ.get("ANCHOR_PASSES", "4")),
        int(os.environ.get("ANCHOR_BT", "6")),
        os.environ.get("ANCHOR_PAIR", "1") == "1",
        os.environ.get("ANCHOR_WARM_CC", "1") == "1",
    )
    if key not in _NC_CACHE:
        _NC_CACHE[key] = build(
            mm_dtype=getattr(mybir.dt, key[0]), passes=key[1], bt=key[2],
            pair=key[3], warm_cc=key[4],
        )
    return _NC_CACHE[key]


def kernel(query, queue_anchor, queue_label):
    nc = _get_nc()
    in_maps = shard_inputs(query, queue_anchor, queue_label)
    res = bass_utils.run_bass_kernel_spmd(
        nc, in_maps, core_ids=list(range(NCORES))
    )
    out = unshard_out([res.results[i]["out"] for i in range(NCORES)])
    return out.astype(np.asarray(queue_label).dtype)


# revision 53
# speedup vs baseline: 1.0145x; 1.0013x over previous
"""Trainium2 Bass kernel for KL-divergence 1-NN label lookup (AnchorStore).

reference:
    self[k]  = mean_d a[k,d]*log a[k,d]
    cross    = einsum('kd,bd->kb', a, log q) / D
    kl[b,k]  = self[k] - cross[k,b]
    out[b]   = queue_label[argmin_k kl[b,k]]

Strategy (8 NeuronCores, D-sharded, fp16 operands):
    Each core owns a D-slice (padded with 1.0 so log()=0 contributes
    nothing), shipped as fp16 in d-tile-major layout [128, NT, K].
    Working in SUM units (scale-invariant for argmin):
        m[b,k] = sum_d lq[d,b]*at[d,k] - sum_d at[d,k]*log(at[d,k])
    K is split into P=4 passes of KW columns; passes 0..P-3 share one
    merged ReduceScatter(add) and pass P-2 gets its own, both fully
    overlapped by compute, leaving only a small RS on the tail.
      - TensorE: stationary lq tiles [128d,128b] x moving at [128d,KW]
        accumulate cross into PSUM; the -self term accumulates via a
        (-1)-stationary x pair-summed t = at*log(at) (DVE adds d-tile
        pairs in fp16 to halve the self-matmul column count); those
        srep matmuls are deferred two batches behind their DVE
        producers to avoid tensor-queue head-of-line blocking.
      - ScalarE computes log() (Ln activation) in large batches.
      - Drain: m = pk + srep -> DRAM -> ReduceScatter.
    Pipeline ramp: pass 0 starts with small d-batches and pass 1's
    first two batches are interleaved into the ramp (they reuse the
    same lq tiles, and their PSUM parity banks are free) so the PE has
    a second stream of ready work while DMA/ACT latency fills.
    Tail: msum loads are pinned behind the last drain (WAW copy) so
    the scheduler can never hoist them; per-slice argmax partials
    (value + label via the is_equal trick) run under the last RS, and
    a 3-column combine emits 32 int32 labels per core; host concats.
"""

import os
import sys

import numpy as np

sys.path.insert(0, "/opt/trn_rl_repo")

from concourse import bacc, bass, mybir, tile  # noqa: E402
from concourse import bass_utils  # noqa: E402

K = 2048
B = 256
D = 50257
NCORES = 8
NT = 50             # d-tiles of 128 per core (padded)
DSH = NT * 128      # 6400
BS = B // NCORES    # 32 queries per core after ReduceScatter
F32 = mybir.dt.float32
F16 = mybir.dt.float16


def build(mm_dtype=F16, passes=4, bt=8, pair=True, warm_cc=True):
    """Build the SPMD Bass graph for one core (all cores identical)."""
    P = passes
    KW = K // P              # k columns per pass
    ncl = KW // 512 if KW >= 512 else 0   # full-512 chunks per pass
    assert KW % 512 == 0 or KW in (256,), KW
    nc = bacc.Bacc(
        "TRN2", target_bir_lowering=False, debug=False, num_devices=NCORES
    )
    # pass-major layout: each (pass, tile-range) batch is one fully
    # contiguous region per partition -> max DMA efficiency
    at_d = nc.dram_tensor(
        "at", [128, P, NT, KW], mm_dtype, kind="ExternalInput"
    )
    qt_d = nc.dram_tensor("qt", [128, NT, B], mm_dtype, kind="ExternalInput")
    lab_d = nc.dram_tensor("lab1", [BS, K], F32, kind="ExternalInput")
    out_d = nc.dram_tensor("out", [BS], mybir.dt.int32, kind="ExternalOutput")

    LN = mybir.ActivationFunctionType.Ln
    AX = mybir.AxisListType.X
    OP = mybir.AluOpType

    # d-tile batches (per pass): groups of `bt` tiles, even-sized for
    # pairs.  Pass 0 ramps up with small batches so the matmul pipeline
    # starts early; later passes use full batches (fewer overheads).
    def mk_batches(ramp):
        out = list(ramp)
        t0 = out[-1][1] if out else 0
        while t0 < NT:
            t1 = min(t0 + bt, NT)
            out.append((t0, t1))
            t0 = t1
        return out

    batches0 = mk_batches([(0, 2), (2, 6), (6, 14)])
    batches_rest = mk_batches([])

    # q chunks for lq computation (front chunks small for fast start)
    qch = [(0, 2), (2, 6), (6, 14), (14, 26), (26, 38), (38, NT)]

    with tile.TileContext(nc) as tc:
        with (
            tc.tile_pool(name="const", bufs=1) as constp,
            tc.tile_pool(name="lqp", bufs=1) as lqp,
            tc.tile_pool(name="qinp", bufs=2) as qinp,
            tc.tile_pool(name="atp", bufs=4) as atp,
            tc.tile_pool(name="latp", bufs=4) as latp,
            tc.tile_pool(name="ttp", bufs=4) as ttp,
            tc.tile_pool(name="tpp", bufs=8) as tpp,
            tc.tile_pool(name="msbp", bufs=2) as msbp,
            tc.tile_pool(name="epp", bufs=1) as epp,
            tc.tile_pool(name="psp", bufs=1, space="PSUM") as psp,
            tc.tile_pool(name="dramp", bufs=1, space="DRAM") as dramp,
        ):
            # --- constants / warmup ---------------------------------
            # Tiny dummy DMAs warm each HWDGE/SWDGE queue so the first
            # real input loads don't pay first-transfer setup latency.
            wdma_d = dramp.tile([1, 16], F32, name="wdma_d", bufs=1)
            for eng in (nc.sync, nc.scalar, nc.gpsimd):
                wdma_s = constp.tile([1, 16], F32, name=f"wdma_{eng.engine}")
                eng.dma_start(wdma_s[:], wdma_d[:])


            # --- lq = log(query^T), fp16, resident -------------------
            lq = lqp.tile([128, NT, B], mm_dtype)
            qsb = []
            for ci, (c0_, c1_) in enumerate(qch):
                qtile = qinp.tile(
                    [128, c1_ - c0_, B], mm_dtype, name=f"qtile_{ci}",
                    tag="qtile",
                )
                qsb.append((qtile, c0_, c1_))
            # All qt chunk DMAs are triggered up front on the (idle)
            # gpsimd queue so every chunk is in flight immediately --
            # the ~5us per-DMA completion latency then overlaps instead
            # of serialising with the lq activations.  The tiny dummy
            # Ln pulls the ACT table load forward, under the qt0 DMA.
            dum = constp.tile([128, 16], F32)
            nc.gpsimd.memset(dum[:], 1.0)
            dumo = constp.tile([128, 16], F32)
            nc.scalar.activation(dumo[:], dum[:], LN)
            negones_f = constp.tile([128, 128], F32)
            nc.gpsimd.memset(negones_f[:], -1.0)
            negones = constp.tile([128, 128], mm_dtype)
            nc.vector.tensor_copy(negones[:], negones_f[:])

            # qt triggers go first on gpsimd: nothing else on that
            # queue is needed before ~20us, and the first matmul waits
            # on qt chunk 0 -> lq.
            for qtile, c0_, c1_ in qsb[:3]:
                nc.gpsimd.dma_start(qtile[:], qt_d[:, c0_:c1_, :])
            nc.scalar.activation(
                lq[:, qch[0][0]:qch[0][1], :], qsb[0][0][:], LN
            )

            # ~40 dummy matmuls into a spare PSUM bank while the PE
            # waits for the first lq tile: the HAM clock gate needs
            # ~3.4us of sustained PE activity to lift the 1.2 GHz cold
            # throttle, so the first real matmuls start at full speed.
            warm_ps = psp.tile([128, 128], F32, name="warm_ps")
            for wi in range(40):
                nc.tensor.matmul(
                    warm_ps[:], negones[:], negones[:],
                    start=(wi == 0), stop=(wi == 39),
                )

            if warm_cc:
                # Tiny dummy collective: pre-warms ncfw/credit state on
                # the CC engine and doubles as the cross-core
                # rendezvous long before the first real ReduceScatter.
                w_in = dramp.tile([1, 64], F32)
                w_out = dramp.tile([NCORES, 64], F32)
                w_sb = constp.tile([1, 64], F32)
                nc.gpsimd.memset(w_sb[:], 1.0)
                nc.gpsimd.dma_start(w_in[:], w_sb[:])
                nc.gpsimd.collective_compute(
                    "AllGather",
                    OP.bypass,
                    replica_groups=[list(range(NCORES))],
                    ins=[w_in.opt()],
                    outs=[w_out.opt()],
                )

            lab1 = constp.tile([BS, K], F32)
            nc.gpsimd.dma_start(lab1[:], lab_d[:])

            # --- PSUM accumulators (parity double-buffered) ----------
            pk = {}
            srep = {}
            for par in range(min(2, P)):
                for bti in range(2):
                    for cl in range(max(1, ncl)):
                        pk[(par, bti, cl)] = psp.tile(
                            [128, min(KW, 512)], F32,
                            name=f"pk_{par}_{bti}_{cl}",
                            tag=f"pk_{par}_{bti}_{cl}",
                        )
                for cl in range(max(1, ncl)):
                    srep[(par, cl)] = psp.tile(
                        [128, min(KW, 512)], F32, name=f"srep_{par}_{cl}",
                        tag=f"srep_{par}_{cl}",
                    )

            # per-slice (value, label) partials, combined at the end:
            # slice 0 = merged passes [0, P-2), slice 1 = pass P-2,
            # slice 2 = pass P-1
            vcat = epp.tile([BS, 3], F32, bufs=1)
            lcat = epp.tile([BS, 3], F32, bufs=1)

            qt_dma_emitted = 1  # chunk 0 already emitted

            # Two collective groups: passes [0, P-1) share one big
            # ReduceScatter (launched after pass P-2, fully overlapped
            # by pass P-1 compute); the last pass gets a small RS on
            # the critical tail.  This keeps the CC engine far from
            # saturation and minimises tail latency.
            KWA = (P - 2) * KW
            ar_a = dramp.tile([B, KWA], F32, name="ar_a", bufs=1)
            rs_a = dramp.tile([BS, KWA], F32, name="rs_a", bufs=1)
            ar_c = dramp.tile([B, KW], F32, name="ar_c", bufs=1)
            rs_c = dramp.tile([BS, KW], F32, name="rs_c", bufs=1)
            ar_b = dramp.tile([B, KW], F32, name="ar_b", bufs=1)
            rs_b = dramp.tile([BS, KW], F32, name="rs_b", bufs=1)

            # Explicit (ps, bi) schedule: pass-1's first two batches
            # are interleaved into pass-0's ramp (they share lq tiles
            # and par=1 PSUM banks are free), so the tensor engine has
            # a second stream of ready work while pass-0's DMA/ACT
            # chain is still filling.
            sched = []
            for ps_i in range(P):
                bl = batches0 if ps_i == 0 else batches_rest
                for bi_i, tb in enumerate(bl):
                    sched.append((ps_i, bi_i, tb, bi_i == len(bl) - 1))
            n0 = len(batches0)
            if P >= 2:
                # move pass-1 batches 0 and 1 up into the pass-0 ramp
                p1b0 = sched.pop(n0)
                p1b1 = sched.pop(n0)
                sched.insert(2, p1b0)
                sched.insert(5, p1b1)

            pend_srep_ps = {ps_i: [] for ps_i in range(P)}
            nclp = max(1, ncl)
            cw = min(KW, 512)
            ramp_i = 0
            for ps, bi, (tb0, tb1), last_b in sched:
                par = ps % 2
                k0 = ps * KW
                pend_srep = pend_srep_ps[ps]

                def flush_srep(fin):
                    tp_, npair_, first_ = pend_srep.pop(0)
                    for i_ in range(npair_):
                        for cl_ in range(nclp):
                            nc.tensor.matmul(
                                srep[(par, cl_)][:],
                                negones[:],
                                tp_[:, i_, cl_ * cw:(cl_ + 1) * cw],
                                start=(first_ and i_ == 0),
                                stop=(fin and i_ == npair_ - 1),
                            )

                if True:
                    n = tb1 - tb0
                    att = atp.tile(
                        [128, n, KW], mm_dtype, name=f"att_{ps}_{bi}",
                        tag="att",
                    )
                    nc.sync.dma_start(
                        att[:], at_d[:, ps, tb0:tb1, :]
                    )
                    # later qt chunks follow the early att batches on
                    # sync so they don't steal HBM bandwidth during the
                    # pipeline ramp; chunk c is triggered one schedule
                    # slot before its lq activation is emitted
                    if ramp_i % 2 == 0 and 3 <= ramp_i // 2 + 2 < len(qsb):
                        qtile, c0_, c1_ = qsb[ramp_i // 2 + 2]
                        nc.sync.dma_start(qtile[:], qt_d[:, c0_:c1_, :])
                    latt = latp.tile(
                        [128, n, KW], mm_dtype,
                        name=f"latt_{ps}_{bi}", tag="latt",
                    )
                    nc.scalar.activation(latt[:], att[:], LN)
                    # trickle in remaining lq activations between the
                    # early batches (their qt DMAs are already in flight)
                    if (
                        qt_dma_emitted < len(qch)
                        and (qt_dma_emitted <= 2
                             or qt_dma_emitted <= ramp_i // 2 + 2)
                    ):
                        qtile, c0_, c1_ = qsb[qt_dma_emitted]
                        nc.scalar.activation(
                            lq[:, c0_:c1_, :], qtile[:], LN
                        )
                        qt_dma_emitted += 1
                    ramp_i += 1
                    tt = ttp.tile(
                        [128, n, KW], mm_dtype, name=f"tt_{ps}_{bi}",
                        tag="tt",
                    )
                    nc.vector.tensor_tensor(tt[:], att[:], latt[:],
                                            op=OP.mult)
                    # before the last batch's cross matmuls, flush all
                    # pending srep matmuls (their pair-sums are long
                    # ready) so only this batch's own self-term work
                    # remains after the final cross matmul
                    if pair and last_b:
                        while pend_srep:
                            flush_srep(False)
                    # cross matmuls
                    for j in range(n):
                        t = tb0 + j
                        for bti in range(2):
                            lhs = lq[:, t, bti * 128:(bti + 1) * 128]
                            for cl in range(nclp):
                                nc.tensor.matmul(
                                    pk[(par, bti, cl)][:],
                                    lhs,
                                    att[:, j, cl * cw:(cl + 1) * cw],
                                    start=(t == 0),
                                    stop=(t == NT - 1),
                                )
                    # self term: pair-sum tt across d-tiles, then matmul.
                    # The srep matmuls for batch bi are emitted after
                    # batch bi+1's cross matmuls (deferred one batch):
                    # if the scalar->DVE chain producing tp lags, the
                    # waiting srep matmul would otherwise head-of-line
                    # block the whole tensor queue.
                    if pair:
                        npair = n // 2
                        tp = tpp.tile(
                            [128, npair, KW], mm_dtype,
                            name=f"tp_{ps}_{bi}", tag="tp",
                        )
                        for i in range(npair):
                            nc.vector.tensor_tensor(
                                tp[:, i, :], tt[:, 2 * i, :],
                                tt[:, 2 * i + 1, :], op=OP.add,
                            )
                        pend_srep.append((tp, npair, bi == 0))

                        if len(pend_srep) > 3:
                            flush_srep(False)
                        if last_b:
                            while len(pend_srep) > 1:
                                flush_srep(False)
                            flush_srep(True)
                    else:
                        for j in range(n):
                            for cl in range(nclp):
                                nc.tensor.matmul(
                                    srep[(par, cl)][:],
                                    negones[:],
                                    tt[:, j, cl * cw:(cl + 1) * cw],
                                    start=(bi == 0 and j == 0),
                                    stop=(last_b and j == n - 1),
                                )

                if ps == 0 and bi == 0:
                    for wi in range(24):
                        nc.tensor.matmul(
                            warm_ps[:], negones[:], negones[:],
                            start=(wi == 0), stop=(wi == 23),
                        )
                if not last_b:
                    continue
                # --- drain pass ps: PSUM -> SBUF -> DRAM -------------
                last_grp = ps == P - 1
                if last_grp:
                    ar_in, koff = ar_b, 0
                elif ps == P - 2:
                    ar_in, koff = ar_c, 0
                else:
                    ar_in, koff = ar_a, ps * KW
                for cl in range(nclp):
                    srep_sb = msbp.tile(
                        [128, cw], F32, name=f"srep_sb_{ps}_{cl}",
                        tag="srep_sb",
                    )
                    nc.vector.tensor_copy(srep_sb[:], srep[(par, cl)][:])
                    for bti in range(2):
                        m_sb = msbp.tile(
                            [128, cw], F32, name=f"m_sb_{ps}_{cl}_{bti}",
                            tag=f"m_sb{bti}",
                        )
                        nc.vector.tensor_tensor(
                            m_sb[:], pk[(par, bti, cl)][:], srep_sb[:],
                            op=OP.add,
                        )
                        nc.gpsimd.dma_start(
                            ar_in[
                                bti * 128:(bti + 1) * 128,
                                koff + cl * cw:koff + (cl + 1) * cw,
                            ],
                            m_sb[:],
                        )
                        m_sb_last = m_sb
                if ps == P - 3:
                    nc.gpsimd.collective_compute(
                        "ReduceScatter",
                        OP.add,
                        replica_groups=[list(range(NCORES))],
                        ins=[ar_a.opt()],
                        outs=[rs_a.opt()],
                    )
                elif ps == P - 2:
                    nc.gpsimd.collective_compute(
                        "ReduceScatter",
                        OP.add,
                        replica_groups=[list(range(NCORES))],
                        ins=[ar_c.opt()],
                        outs=[rs_c.opt()],
                    )
                elif last_grp:
                    nc.gpsimd.collective_compute(
                        "ReduceScatter",
                        OP.add,
                        replica_groups=[list(range(NCORES))],
                        ins=[ar_b.opt()],
                        outs=[rs_b.opt()],
                    )
            # --- tail: msum loads + per-slice epilogues -------------
            # Each msum load is pinned behind the last pass's drain via
            # a tiny WAW copy so the Tile scheduler can never hoist it
            # (or the epilogue that reads it) into the main pipeline --
            # if a ReduceScatter finishes late, nothing upstream stalls.
            # They ride the sync queue, idle once att loads are done.
            msum_a = epp.tile([BS, KWA], F32, bufs=1)
            msum_c = epp.tile([BS, KW], F32, bufs=1)
            msum_b = epp.tile([BS, KW], F32, bufs=1)
            for mt in (msum_a, msum_c, msum_b):
                nc.vector.tensor_copy(mt[0:BS, 0:1], m_sb_last[0:BS, 0:1])
            nc.sync.dma_start(msum_a[:], rs_a[:])
            nc.sync.dma_start(msum_c[:], rs_c[:])
            nc.sync.dma_start(msum_b[:], rs_b[:])

            def emit_epi(col, msum, w, lab_off):
                # argmax partials (value + label) over one k-slice
                nc.vector.tensor_reduce(
                    vcat[:, col:col + 1], msum[:], axis=AX, op=OP.max
                )
                cand = epp.tile(
                    [BS, w], F32, name=f"cand_{col}", tag=f"cand_{col}",
                )
                nc.vector.scalar_tensor_tensor(
                    cand[:], msum[:], vcat[:, col:col + 1],
                    lab1[:, lab_off:lab_off + w],
                    op0=OP.is_equal, op1=OP.mult,
                )
                nc.vector.tensor_reduce(
                    lcat[:, col:col + 1], cand[:], axis=AX, op=OP.max
                )

            emit_epi(0, msum_a, KWA, 0)
            emit_epi(1, msum_c, KW, (P - 2) * KW)
            emit_epi(2, msum_b, KW, (P - 1) * KW)

            # --- final combine across passes ------------------------
            vg = epp.tile([BS, 1], F32, bufs=1)
            nc.vector.tensor_reduce(vg[:], vcat[:], axis=AX, op=OP.max)
            candp = epp.tile([BS, 3], F32, bufs=1)
            nc.vector.scalar_tensor_tensor(
                candp[:], vcat[:], vg[:], lcat[:],
                op0=OP.is_equal, op1=OP.mult,
            )
            lmax = epp.tile([BS, 1], F32, bufs=1)
            nc.vector.tensor_reduce(lmax[:], candp[:], axis=AX, op=OP.max)
            labf = epp.tile([BS, 1], F32, bufs=1)
            nc.vector.tensor_scalar_add(labf[:], lmax[:], -1.0)
            labi = epp.tile([BS, 1], mybir.dt.int32, bufs=1)
            nc.vector.tensor_copy(labi[:], labf[:])
            nc.scalar.dma_start(out_d[:], labi[:])

    nc.compile()
    return nc


def shard_inputs(query, queue_anchor, queue_label, dsh=DSH, d_real=D,
                 passes=None):
    """Host-side layout prep: pad D with 1.0 (log 1 = 0); at in
    pass-major fp16 layout [128, P, NT, KW], qt tile-major
    [128, NT, B]; label row replicated."""
    if passes is None:
        passes = int(os.environ.get("ANCHOR_PASSES", "4"))
    kw = K // passes
    np_dt = np.float16
    q = np.asarray(query, np.float32)
    a = np.asarray(queue_anchor, np.float32)
    lab1 = (np.asarray(queue_label).astype(np.float32) + 1.0)[None, :]
    lab1 = np.ascontiguousarray(np.broadcast_to(lab1, (BS, lab1.shape[1])))
    in_maps = []
    for c in range(NCORES):
        lo = c * dsh
        hi = min((c + 1) * dsh, d_real)
        at = np.ones((dsh, a.shape[0]), np_dt)
        qt = np.ones((dsh, q.shape[0]), np_dt)
        if hi > lo:
            at[: hi - lo, :] = a[:, lo:hi].T.astype(np_dt)
            qt[: hi - lo, :] = q[:, lo:hi].T.astype(np_dt)
        # at: [dsh, K] -> [128, P, NT, KW] (pass-major, contiguous)
        at = np.ascontiguousarray(
            at.reshape(NT, 128, passes, kw).transpose(1, 2, 0, 3)
        )
        # qt: [dsh, B] -> tile-major [128, NT, B]
        qt = np.ascontiguousarray(
            qt.reshape(NT, 128, -1).transpose(1, 0, 2)
        )
        in_maps.append({"at": at, "qt": qt, "lab1": lab1})
    return in_maps


def unshard_out(per_core_outs, split_rs=False):
    """Reassemble the 8 cores' 32-label slices into the [256] output."""
    return np.concatenate([np.asarray(o) for o in per_core_outs])


_NC_CACHE = {}


def _split_rs_active():
    return False


def _get_nc():
    key = (
        os.environ.get("ANCHOR_MM_DTYPE", "float16"),
        int(os.environ.get("ANCHOR_PASSES", "4")),
        int(os.environ.get("ANCHOR_BT", "6")),
        os.environ.get("ANCHOR_PAIR", "1") == "1",
        os.environ.get("ANCHOR_WARM_CC", "1") == "1",
    )
    if key not in _NC_CACHE:
        _NC_CACHE[key] = build(
            mm_dtype=getattr(mybir.dt, key[0]), passes=key[1], bt=key[2],
            pair=key[3], warm_cc=key[4],
        )
    return _NC_CACHE[key]


def kernel(query, queue_anchor, queue_label):
    nc = _get_nc()
    in_maps = shard_inputs(query, queue_anchor, queue_label)
    res = bass_utils.run_bass_kernel_spmd(
        nc, in_maps, core_ids=list(range(NCORES))
    )
    out = unshard_out([res.results[i]["out"] for i in range(NCORES)])
    return out.astype(np.asarray(queue_label).dtype)


# revision 55
# speedup vs baseline: 1.0273x; 1.0127x over previous
"""Trainium2 Bass kernel for KL-divergence 1-NN label lookup (AnchorStore).

reference:
    self[k]  = mean_d a[k,d]*log a[k,d]
    cross    = einsum('kd,bd->kb', a, log q) / D
    kl[b,k]  = self[k] - cross[k,b]
    out[b]   = queue_label[argmin_k kl[b,k]]

Strategy (8 NeuronCores, D-sharded, fp16 operands):
    Each core owns a D-slice (padded with 1.0 so log()=0 contributes
    nothing), shipped as fp16 in d-tile-major layout [128, NT, K].
    Working in SUM units (scale-invariant for argmin):
        m[b,k] = sum_d lq[d,b]*at[d,k] - sum_d at[d,k]*log(at[d,k])
    K is split into P=4 passes of KW columns; passes 0..P-3 share one
    merged ReduceScatter(add) and pass P-2 gets its own, both fully
    overlapped by compute, leaving only a small RS on the tail.
      - TensorE: stationary lq tiles [128d,128b] x moving at [128d,KW]
        accumulate cross into PSUM; the -self term accumulates via a
        (-1)-stationary x pair-summed t = at*log(at) (DVE adds d-tile
        pairs in fp16 to halve the self-matmul column count); those
        srep matmuls are deferred two batches behind their DVE
        producers to avoid tensor-queue head-of-line blocking.
      - ScalarE computes log() (Ln activation) in large batches.
      - Drain: m = pk + srep -> DRAM -> ReduceScatter.
    Pipeline ramp: pass 0 starts with small d-batches and pass 1's
    first two batches are interleaved into the ramp (they reuse the
    same lq tiles, and their PSUM parity banks are free) so the PE has
    a second stream of ready work while DMA/ACT latency fills.
    Tail: msum loads are pinned behind the last drain (WAW copy) so
    the scheduler can never hoist them; per-slice argmax partials
    (value + label via the is_equal trick) run under the last RS, and
    a 3-column combine emits 32 int32 labels per core; host concats.
"""

import os
import sys

import numpy as np

sys.path.insert(0, "/opt/trn_rl_repo")

from concourse import bacc, bass, mybir, tile  # noqa: E402
from concourse import bass_utils  # noqa: E402

K = 2048
B = 256
D = 50257
NCORES = 8
NT = 50             # d-tiles of 128 per core (padded)
DSH = NT * 128      # 6400
BS = B // NCORES    # 32 queries per core after ReduceScatter
F32 = mybir.dt.float32
F16 = mybir.dt.float16


def build(mm_dtype=F16, passes=4, bt=8, pair=True, warm_cc=True):
    """Build the SPMD Bass graph for one core (all cores identical)."""
    P = passes
    KW = K // P              # k columns per pass
    ncl = KW // 512 if KW >= 512 else 0   # full-512 chunks per pass
    assert KW % 512 == 0 or KW in (256,), KW
    nc = bacc.Bacc(
        "TRN2", target_bir_lowering=False, debug=False, num_devices=NCORES
    )
    # pass-major layout: each (pass, tile-range) batch is one fully
    # contiguous region per partition -> max DMA efficiency
    at_d = nc.dram_tensor(
        "at", [128, P, NT, KW], mm_dtype, kind="ExternalInput"
    )
    qt_d = nc.dram_tensor("qt", [128, NT, B], mm_dtype, kind="ExternalInput")
    lab_d = nc.dram_tensor("lab1", [BS, K], F32, kind="ExternalInput")
    out_d = nc.dram_tensor("out", [BS], mybir.dt.int32, kind="ExternalOutput")

    LN = mybir.ActivationFunctionType.Ln
    AX = mybir.AxisListType.X
    OP = mybir.AluOpType

    # d-tile batches (per pass): groups of `bt` tiles, even-sized for
    # pairs.  Pass 0 ramps up with small batches so the matmul pipeline
    # starts early; later passes use full batches (fewer overheads).
    def mk_batches(ramp):
        out = list(ramp)
        t0 = out[-1][1] if out else 0
        while t0 < NT:
            t1 = min(t0 + bt, NT)
            out.append((t0, t1))
            t0 = t1
        return out

    batches0 = mk_batches([(0, 2), (2, 6), (6, 14)])
    batches_rest = mk_batches([])

    # q chunks for lq computation (front chunks small for fast start)
    qch = [(0, 2), (2, 6), (6, 14), (14, 26), (26, 38), (38, NT)]

    with tile.TileContext(nc) as tc:
        with (
            tc.tile_pool(name="const", bufs=1) as constp,
            tc.tile_pool(name="lqp", bufs=1) as lqp,
            tc.tile_pool(name="qinp", bufs=2) as qinp,
            tc.tile_pool(name="atp", bufs=4) as atp,
            tc.tile_pool(name="latp", bufs=4) as latp,
            tc.tile_pool(name="ttp", bufs=4) as ttp,
            tc.tile_pool(name="tpp", bufs=8) as tpp,
            tc.tile_pool(name="msbp", bufs=2) as msbp,
            tc.tile_pool(name="epp", bufs=1) as epp,
            tc.tile_pool(name="psp", bufs=1, space="PSUM") as psp,
            tc.tile_pool(name="dramp", bufs=1, space="DRAM") as dramp,
        ):
            # --- constants / warmup ---------------------------------
            # Tiny dummy DMAs warm each HWDGE/SWDGE queue so the first
            # real input loads don't pay first-transfer setup latency.
            wdma_d = dramp.tile([1, 16], F32, name="wdma_d", bufs=1)
            for eng in (nc.sync, nc.scalar, nc.gpsimd):
                wdma_s = constp.tile([1, 16], F32, name=f"wdma_{eng.engine}")
                eng.dma_start(wdma_s[:], wdma_d[:])


            # --- lq = log(query^T), fp16, resident -------------------
            lq = lqp.tile([128, NT, B], mm_dtype)
            qsb = []
            for ci, (c0_, c1_) in enumerate(qch):
                qtile = qinp.tile(
                    [128, c1_ - c0_, B], mm_dtype, name=f"qtile_{ci}",
                    tag="qtile",
                )
                qsb.append((qtile, c0_, c1_))
            # All qt chunk DMAs are triggered up front on the (idle)
            # gpsimd queue so every chunk is in flight immediately --
            # the ~5us per-DMA completion latency then overlaps instead
            # of serialising with the lq activations.  The tiny dummy
            # Ln pulls the ACT table load forward, under the qt0 DMA.
            dum = constp.tile([128, 16], F32)
            nc.gpsimd.memset(dum[:], 1.0)
            dumo = constp.tile([128, 16], F32)
            nc.scalar.activation(dumo[:], dum[:], LN)
            negones_f = constp.tile([128, 128], F32)
            nc.gpsimd.memset(negones_f[:], -1.0)
            negones = constp.tile([128, 128], mm_dtype)
            nc.vector.tensor_copy(negones[:], negones_f[:])

            # qt triggers go first on gpsimd: nothing else on that
            # queue is needed before ~20us, and the first matmul waits
            # on qt chunk 0 -> lq.
            for qtile, c0_, c1_ in qsb[:3]:
                nc.gpsimd.dma_start(qtile[:], qt_d[:, c0_:c1_, :])
            nc.scalar.activation(
                lq[:, qch[0][0]:qch[0][1], :], qsb[0][0][:], LN
            )

            # ~40 dummy matmuls into a spare PSUM bank while the PE
            # waits for the first lq tile: the HAM clock gate needs
            # ~3.4us of sustained PE activity to lift the 1.2 GHz cold
            # throttle, so the first real matmuls start at full speed.
            warm_ps = psp.tile([128, 128], F32, name="warm_ps")
            for wi in range(40):
                nc.tensor.matmul(
                    warm_ps[:], negones[:], negones[:],
                    start=(wi == 0), stop=(wi == 39),
                )

            if warm_cc:
                # Tiny dummy collective: pre-warms ncfw/credit state on
                # the CC engine and doubles as the cross-core
                # rendezvous long before the first real ReduceScatter.
                w_in = dramp.tile([1, 64], F32)
                w_out = dramp.tile([NCORES, 64], F32)
                w_sb = constp.tile([1, 64], F32)
                nc.gpsimd.memset(w_sb[:], 1.0)
                nc.gpsimd.dma_start(w_in[:], w_sb[:])
                nc.gpsimd.collective_compute(
                    "AllGather",
                    OP.bypass,
                    replica_groups=[list(range(NCORES))],
                    ins=[w_in.opt()],
                    outs=[w_out.opt()],
                )

            lab1 = constp.tile([BS, K], F32)
            nc.gpsimd.dma_start(lab1[:], lab_d[:])

            # --- PSUM accumulators (parity double-buffered) ----------
            pk = {}
            srep = {}
            for par in range(min(2, P)):
                for bti in range(2):
                    for cl in range(max(1, ncl)):
                        pk[(par, bti, cl)] = psp.tile(
                            [128, min(KW, 512)], F32,
                            name=f"pk_{par}_{bti}_{cl}",
                            tag=f"pk_{par}_{bti}_{cl}",
                        )
                for cl in range(max(1, ncl)):
                    srep[(par, cl)] = psp.tile(
                        [128, min(KW, 512)], F32, name=f"srep_{par}_{cl}",
                        tag=f"srep_{par}_{cl}",
                    )

            # per-slice (value, label) partials, combined at the end:
            # slice 0 = merged passes [0, P-2), slice 1 = pass P-2,
            # slice 2 = pass P-1
            vcat = epp.tile([BS, 3], F32, bufs=1)
            lcat = epp.tile([BS, 3], F32, bufs=1)

            qt_dma_emitted = 1  # chunk 0 already emitted

            # Two collective groups: passes [0, P-1) share one big
            # ReduceScatter (launched after pass P-2, fully overlapped
            # by pass P-1 compute); the last pass gets a small RS on
            # the critical tail.  This keeps the CC engine far from
            # saturation and minimises tail latency.
            KWA = (P - 2) * KW
            ar_a = dramp.tile([B, KWA], F32, name="ar_a", bufs=1)
            rs_a = dramp.tile([BS, KWA], F32, name="rs_a", bufs=1)
            ar_c = dramp.tile([B, KW], F32, name="ar_c", bufs=1)
            rs_c = dramp.tile([BS, KW], F32, name="rs_c", bufs=1)
            ar_b = dramp.tile([B, KW], F32, name="ar_b", bufs=1)
            rs_b = dramp.tile([BS, KW], F32, name="rs_b", bufs=1)

            # Explicit (ps, bi) schedule: pass-1's first two batches
            # are interleaved into pass-0's ramp (they share lq tiles
            # and par=1 PSUM banks are free), so the tensor engine has
            # a second stream of ready work while pass-0's DMA/ACT
            # chain is still filling.
            sched = []
            for ps_i in range(P):
                bl = batches0 if ps_i == 0 else batches_rest
                for bi_i, tb in enumerate(bl):
                    sched.append((ps_i, bi_i, tb, bi_i == len(bl) - 1))
            n0 = len(batches0)
            if P >= 2:
                # move pass-1 batches 0 and 1 up into the pass-0 ramp
                p1b0 = sched.pop(n0)
                p1b1 = sched.pop(n0)
                sched.insert(2, p1b0)
                sched.insert(5, p1b1)

            pend_srep_ps = {ps_i: [] for ps_i in range(P)}
            nclp = max(1, ncl)
            cw = min(KW, 512)
            ramp_i = 0
            for ps, bi, (tb0, tb1), last_b in sched:
                par = ps % 2
                k0 = ps * KW
                pend_srep = pend_srep_ps[ps]

                def flush_srep(fin):
                    tp_, npair_, first_ = pend_srep.pop(0)
                    for i_ in range(npair_):
                        for cl_ in range(nclp):
                            nc.tensor.matmul(
                                srep[(par, cl_)][:],
                                negones[:],
                                tp_[:, i_, cl_ * cw:(cl_ + 1) * cw],
                                start=(first_ and i_ == 0),
                                stop=(fin and i_ == npair_ - 1),
                            )

                if True:
                    n = tb1 - tb0
                    att = atp.tile(
                        [128, n, KW], mm_dtype, name=f"att_{ps}_{bi}",
                        tag="att",
                    )
                    nc.sync.dma_start(
                        att[:], at_d[:, ps, tb0:tb1, :]
                    )
                    # later qt chunks follow the early att batches on
                    # sync so they don't steal HBM bandwidth during the
                    # pipeline ramp; chunk c is triggered one schedule
                    # slot before its lq activation is emitted
                    if ramp_i % 2 == 0 and 3 <= ramp_i // 2 + 2 < len(qsb):
                        qtile, c0_, c1_ = qsb[ramp_i // 2 + 2]
                        nc.sync.dma_start(qtile[:], qt_d[:, c0_:c1_, :])
                    latt = latp.tile(
                        [128, n, KW], mm_dtype,
                        name=f"latt_{ps}_{bi}", tag="latt",
                    )
                    nc.scalar.activation(latt[:], att[:], LN)
                    # trickle in remaining lq activations between the
                    # early batches (their qt DMAs are already in flight)
                    if (
                        qt_dma_emitted < len(qch)
                        and (qt_dma_emitted <= 2
                             or qt_dma_emitted <= ramp_i // 2 + 2)
                    ):
                        qtile, c0_, c1_ = qsb[qt_dma_emitted]
                        nc.scalar.activation(
                            lq[:, c0_:c1_, :], qtile[:], LN
                        )
                        qt_dma_emitted += 1
                    ramp_i += 1
                    tt = ttp.tile(
                        [128, n, KW], mm_dtype, name=f"tt_{ps}_{bi}",
                        tag="tt",
                    )
                    nc.vector.tensor_tensor(tt[:], att[:], latt[:],
                                            op=OP.mult)
                    # before the last batch's cross matmuls, flush all
                    # pending srep matmuls (their pair-sums are long
                    # ready) so only this batch's own self-term work
                    # remains after the final cross matmul
                    if pair and last_b:
                        while pend_srep:
                            flush_srep(False)
                    # cross matmuls
                    for j in range(n):
                        t = tb0 + j
                        for bti in range(2):
                            lhs = lq[:, t, bti * 128:(bti + 1) * 128]
                            for cl in range(nclp):
                                nc.tensor.matmul(
                                    pk[(par, bti, cl)][:],
                                    lhs,
                                    att[:, j, cl * cw:(cl + 1) * cw],
                                    start=(t == 0),
                                    stop=(t == NT - 1),
                                )
                    # self term: pair-sum tt across d-tiles, then matmul.
                    # The srep matmuls for batch bi are emitted after
                    # batch bi+1's cross matmuls (deferred one batch):
                    # if the scalar->DVE chain producing tp lags, the
                    # waiting srep matmul would otherwise head-of-line
                    # block the whole tensor queue.
                    if pair:
                        npair = n // 2
                        tp = tpp.tile(
                            [128, npair, KW], mm_dtype,
                            name=f"tp_{ps}_{bi}", tag="tp",
                        )
                        for i in range(npair):
                            nc.vector.tensor_tensor(
                                tp[:, i, :], tt[:, 2 * i, :],
                                tt[:, 2 * i + 1, :], op=OP.add,
                            )
                        pend_srep.append((tp, npair, bi == 0))

                        if len(pend_srep) > 3:
                            flush_srep(False)
                        if last_b:
                            while len(pend_srep) > 1:
                                flush_srep(False)
                            flush_srep(True)
                    else:
                        for j in range(n):
                            for cl in range(nclp):
                                nc.tensor.matmul(
                                    srep[(par, cl)][:],
                                    negones[:],
                                    tt[:, j, cl * cw:(cl + 1) * cw],
                                    start=(bi == 0 and j == 0),
                                    stop=(last_b and j == n - 1),
                                )

                if ps == 0 and bi == 0:
                    for wi in range(24):
                        nc.tensor.matmul(
                            warm_ps[:], negones[:], negones[:],
                            start=(wi == 0), stop=(wi == 23),
                        )
                if not last_b:
                    continue
                # --- drain pass ps: PSUM -> SBUF -> DRAM -------------
                last_grp = ps == P - 1
                if last_grp:
                    ar_in, koff = ar_b, 0
                elif ps == P - 2:
                    ar_in, koff = ar_c, 0
                else:
                    ar_in, koff = ar_a, ps * KW
                for cl in range(nclp):
                    srep_sb = msbp.tile(
                        [128, cw], F32, name=f"srep_sb_{ps}_{cl}",
                        tag="srep_sb",
                    )
                    nc.vector.tensor_copy(srep_sb[:], srep[(par, cl)][:])
                    for bti in range(2):
                        m_sb = msbp.tile(
                            [128, cw], F32, name=f"m_sb_{ps}_{cl}_{bti}",
                            tag=f"m_sb{bti}",
                        )
                        nc.vector.tensor_tensor(
                            m_sb[:], pk[(par, bti, cl)][:], srep_sb[:],
                            op=OP.add,
                        )
                        nc.gpsimd.dma_start(
                            ar_in[
                                bti * 128:(bti + 1) * 128,
                                koff + cl * cw:koff + (cl + 1) * cw,
                            ],
                            m_sb[:],
                        )
                        m_sb_last = m_sb
                if ps == P - 3:
                    nc.gpsimd.collective_compute(
                        "ReduceScatter",
                        OP.add,
                        replica_groups=[list(range(NCORES))],
                        ins=[ar_a.opt()],
                        outs=[rs_a.opt()],
                    )
                elif ps == P - 2:
                    nc.gpsimd.collective_compute(
                        "ReduceScatter",
                        OP.add,
                        replica_groups=[list(range(NCORES))],
                        ins=[ar_c.opt()],
                        outs=[rs_c.opt()],
                    )
                elif last_grp:
                    nc.gpsimd.collective_compute(
                        "ReduceScatter",
                        OP.add,
                        replica_groups=[list(range(NCORES))],
                        ins=[ar_b.opt()],
                        outs=[rs_b.opt()],
                    )
            # --- tail: msum loads + per-slice epilogues -------------
            # Each msum load is pinned behind the last pass's drain via
            # a tiny WAW copy so the Tile scheduler can never hoist it
            # (or the epilogue that reads it) into the main pipeline --
            # if a ReduceScatter finishes late, nothing upstream stalls.
            # They ride the sync queue, idle once att loads are done.
            msum_a = epp.tile([BS, KWA], F32, bufs=1)
            msum_c = epp.tile([BS, KW], F32, bufs=1)
            msum_b = epp.tile([BS, KW], F32, bufs=1)
            for mt in (msum_a, msum_c, msum_b):
                nc.vector.tensor_copy(mt[0:BS, 0:1], m_sb_last[0:BS, 0:1])
            nc.sync.dma_start(msum_a[:], rs_a[:])
            nc.sync.dma_start(msum_c[:], rs_c[:])
            nc.sync.dma_start(msum_b[:], rs_b[:])

            def emit_epi(col, msum, w, lab_off):
                # argmax partials (value + label) over one k-slice
                nc.vector.tensor_reduce(
                    vcat[:, col:col + 1], msum[:], axis=AX, op=OP.max
                )
                cand = epp.tile(
                    [BS, w], F32, name=f"cand_{col}", tag=f"cand_{col}",
                )
                nc.vector.scalar_tensor_tensor(
                    cand[:], msum[:], vcat[:, col:col + 1],
                    lab1[:, lab_off:lab_off + w],
                    op0=OP.is_equal, op1=OP.mult,
                )
                nc.vector.tensor_reduce(
                    lcat[:, col:col + 1], cand[:], axis=AX, op=OP.max
                )

            emit_epi(0, msum_a, KWA, 0)
            emit_epi(1, msum_c, KW, (P - 2) * KW)
            emit_epi(2, msum_b, KW, (P - 1) * KW)

            # --- final combine across passes ------------------------
            vg = epp.tile([BS, 1], F32, bufs=1)
            nc.vector.tensor_reduce(vg[:], vcat[:], axis=AX, op=OP.max)
            candp = epp.tile([BS, 3], F32, bufs=1)
            nc.vector.scalar_tensor_tensor(
                candp[:], vcat[:], vg[:], lcat[:],
                op0=OP.is_equal, op1=OP.mult,
            )
            lmax = epp.tile([BS, 1], F32, bufs=1)
            nc.vector.tensor_reduce(lmax[:], candp[:], axis=AX, op=OP.max)
            labf = epp.tile([BS, 1], F32, bufs=1)
            nc.vector.tensor_scalar_add(labf[:], lmax[:], -1.0)
            labi = epp.tile([BS, 1], mybir.dt.int32, bufs=1)
            nc.vector.tensor_copy(labi[:], labf[:])
            nc.scalar.dma_start(out_d[:], labi[:])

    nc.compile()
    return nc


def shard_inputs(query, queue_anchor, queue_label, dsh=DSH, d_real=D,
                 passes=None):
    """Host-side layout prep: pad D with 1.0 (log 1 = 0); at in
    pass-major fp16 layout [128, P, NT, KW], qt tile-major
    [128, NT, B]; label row replicated."""
    if passes is None:
        passes = int(os.environ.get("ANCHOR_PASSES", "4"))
    kw = K // passes
    np_dt = np.float16
    q = np.asarray(query, np.float32)
    a = np.asarray(queue_anchor, np.float32)
    lab1 = (np.asarray(queue_label).astype(np.float32) + 1.0)[None, :]
    lab1 = np.ascontiguousarray(np.broadcast_to(lab1, (BS, lab1.shape[1])))
    in_maps = []
    for c in range(NCORES):
        lo = c * dsh
        hi = min((c + 1) * dsh, d_real)
        at = np.ones((dsh, a.shape[0]), np_dt)
        qt = np.ones((dsh, q.shape[0]), np_dt)
        if hi > lo:
            at[: hi - lo, :] = a[:, lo:hi].T.astype(np_dt)
            qt[: hi - lo, :] = q[:, lo:hi].T.astype(np_dt)
        # at: [dsh, K] -> [128, P, NT, KW] (pass-major, contiguous)
        at = np.ascontiguousarray(
            at.reshape(NT, 128, passes, kw).transpose(1, 2, 0, 3)
        )
        # qt: [dsh, B] -> tile-major [128, NT, B]
        qt = np.ascontiguousarray(
            qt.reshape(NT, 128, -1).transpose(1, 0, 2)
        )
        in_maps.append({"at": at, "qt": qt, "lab1": lab1})
    return in_maps


def unshard_out(per_core_outs, split_rs=False):
    """Reassemble the 8 cores' 32-label slices into the [256] output."""
    return np.concatenate([np.asarray(o) for o in per_core_outs])


_NC_CACHE = {}


def _split_rs_active():
    return False


def _get_nc():
    key = (
        os.environ.get("ANCHOR_MM_DTYPE", "float16"),
        int(os.environ.get("ANCHOR_PASSES", "4")),
        int(os.environ.get("ANCHOR_BT", "6")),
        os.environ.get("ANCHOR_PAIR", "1") == "1",
        os.environ.get("ANCHOR_WARM_CC", "1") == "1",
    )
    if key not in _NC_CACHE:
        _NC_CACHE[key] = build(
            mm_dtype=getattr(mybir.dt, key[0]), passes=key[1], bt=key[2],
            pair=key[3], warm_cc=key[4],
        )
    return _NC_CACHE[key]


def kernel(query, queue_anchor, queue_label):
    nc = _get_nc()
    in_maps = shard_inputs(query, queue_anchor, queue_label)
    res = bass_utils.run_bass_kernel_spmd(
        nc, in_maps, core_ids=list(range(NCORES))
    )
    out = unshard_out([res.results[i]["out"] for i in range(NCORES)])
    return out.astype(np.asarray(queue_label).dtype)
